# revision 31
# baseline (speedup 1.0000x reference)
"""Trainium2 Bass kernel for 2-layer HGT message passing + sparse gather-dot,
sharded over 8 NeuronCores.

Layout strategy (v3):
 - Nodes of each type are RELABELED host-side by in-degree rank:
   new_id = band*128 + slot, band = rank//128 (80 bands, degree-sorted),
   core(band) = band % 8.  All indices (edges, final queries) are remapped
   through the permutation, so the device never sees it.
 - Edge phase uses a dst-per-partition layout: for a 128-dst block, slot
   (p, j) holds the j-th in-edge of dst p.  J_b = max in-block degree is a
   compile-time constant per block.  Blocks run in descending-J order so
   the heavy block's gathers prefetch first and the pipeline tail is the
   smallest block.
 - gelu+update tails are deferred to the end of each direction so the
   Activation engine swaps tables (Exp<->Gelu) only twice per direction.
 - The per-type AllGather writes DIRECTLY into the emed staging table via
   a strided output AP (no stage readback / emed rewrite).
 - Final gather-dot: queries are grouped by their m-node's OWNER core and
   packed into query-count-sorted 128-node tiles (slot (p, j) = j-th query
   of m-node p).  The dense Em side comes from a core-LOCAL table (both
   layers' outputs, written without any collective), so the layer-1 type-1
   AllGather is not needed at all (3 collectives, not 4).  Only Ed rows
   are gathered per query (from emed2, gated on the EARLY layer-1
   collective) -> half the final gather traffic of the pair-gather scheme
   and no collective on the final critical path.
 - All tables and gathered data are bf16 (512B gather rows).  PSUM stays
   f32.
"""
import numpy as np

N = 10000
NP = 10240          # padded node count (80 tiles of 128)
NT = NP // 128      # 80 tiles
NCORE = 8
NBLK = NT // NCORE  # 10 blocks (dst tiles) per core
NLOC = NBLK * 128   # 1280 nodes owned per core
F = 128; HID = 128; H = 8; D = 16; L = 2
EF = 500000
NFT = NBLK          # 10 final m-tiles per core
ZROW = NP           # zero row in kv/emed tables used by padding slots


def _wrap_idx(idx):
    """int index list (len%16==0) -> [128, len//16] int16 in gather format."""
    a = np.asarray(idx, np.int16).reshape(-1, 16).T
    return np.ascontiguousarray(np.tile(a, (8, 1)))


def _blockdiag(a):
    out = np.zeros((HID, HID), np.float32)
    for h in range(H):
        out[h * D:(h + 1) * D, h * D:(h + 1) * D] = a[h]
    return out


# column permutation (h,d) -> d-major (d*8+h)
_PDH = np.zeros(HID, np.int64)
for _h in range(H):
    for _d in range(D):
        _PDH[_d * H + _h] = _h * D + _d   # new col i=d*8+h takes old col h*16+d


def _perm_from_degree(deg):
    """deg[NP] -> perm (old->new), degree-ascending bands dealt round-robin."""
    order = np.argsort(deg, kind="stable")       # order[r] = old id
    perm = np.empty(NP, np.int64)
    r = np.arange(NP)
    perm[order] = r                               # new_id = rank
    return perm


def _rmap(x):
    """permuted id -> table row in AllGather output order (r, b, p)."""
    band = x // 128
    return ((band % NCORE) * NBLK + band // NCORE) * 128 + x % 128


_RINV = None


def _rinv():
    global _RINV
    if _RINV is None:
        inv = np.empty(NP, np.int64)
        inv[_rmap(np.arange(NP))] = np.arange(NP)
        _RINV = inv
    return _RINV


def _prep_edges(ei, perm_s, perm_d):
    """-> per-core dict(idx [128, SJ*8] i16, padc [NBLK,128] f32), J list."""
    s = _rmap(perm_s[np.asarray(ei[0])])
    d = perm_d[np.asarray(ei[1])]
    band = d // 128
    core = band % NCORE
    blk = band // NCORE
    p = d % 128
    # j-th edge of each dst: stable sort by d, position within group
    order = np.argsort(d, kind="stable")
    ds = d[order]
    cnt = np.bincount(d, minlength=NP)
    starts = np.zeros(NP + 1, np.int64)
    np.cumsum(cnt, out=starts[1:])
    j_of = np.arange(len(ds)) - starts[ds]
    # J per (core, blk): max degree in band
    J = np.zeros((NCORE, NBLK), np.int64)
    for b in range(NT):
        mx = cnt[b * 128:(b + 1) * 128].max()
        J[b % NCORE, b // NCORE] = max(J[b % NCORE, b // NCORE], mx)
    Jb = [max(1, int(J[:, b].max())) for b in range(NBLK)]  # same for all cores
    out = []
    ss = s[order]
    cs = core[order]; bs = blk[order]; ps = p[order]
    for c in range(NCORE):
        idxs = []
        padc = np.zeros((NBLK, 128), np.float32)
        m_c = cs == c
        for b in range(NBLK):
            Jcb = Jb[b]
            A = np.full((Jcb, 128), ZROW, np.int64)
            m = m_c & (bs == b)
            A[j_of[m], ps[m]] = ss[m]
            band_cnt = cnt[(b * NCORE + c) * 128:(b * NCORE + c + 1) * 128]
            # 1e-3 denominator bias keeps zero-degree rows finite (0*1000=0);
            # relative effect on real weights ~1e-3/32, far under tolerance
            padc[b, :] = (Jcb - band_cnt).astype(np.float32) - 1e-3
            idxs.append(_wrap_idx(A.reshape(-1)))
        out.append({"idx": np.ascontiguousarray(np.hstack(idxs)),
                    "padc": padc})
    return out, Jb


def _prep_final(eidx, perm1, perm2):
    """Queries grouped by m-node owner core; count-sorted local tiles.

    Returns per-core dicts (emlid: local m ids per tile for the dense Em
    gather, fei: ed gather idx per slot, pos: slot -> original query id or
    -1) and JF (per-tile max query count, shared across cores).
    """
    mi = perm1[np.asarray(eidx[0])]
    di = perm2[np.asarray(eidx[1])]
    band = mi // 128
    cq = band % NCORE
    lid = (band // NCORE) * 128 + mi % 128       # local row in xtloc
    # ed side reads the emed2c table whose rows are in AllGather output
    # order (r, b, p)
    di = _rmap(di)
    key = cq * NLOC + lid
    cntq = np.bincount(key, minlength=NCORE * NLOC).reshape(NCORE, NLOC)
    rk = np.empty((NCORE, NLOC), np.int64)
    sorted_cnt = np.empty((NCORE, NLOC), np.int64)
    emlid = np.empty((NCORE, NLOC), np.int64)
    for c in range(NCORE):
        o = np.argsort(-cntq[c], kind="stable")
        rk[c, o] = np.arange(NLOC)
        sorted_cnt[c] = cntq[c][o]
        emlid[c] = o                              # rank -> local id
    JF = [max(1, int(sorted_cnt[:, g * 128].max())) for g in range(NFT)]
    # position of each query within its (core, m-node) group
    order = np.argsort(key, kind="stable")
    ks = key[order]
    starts = np.zeros(NCORE * NLOC + 1, np.int64)
    np.cumsum(cntq.reshape(-1), out=starts[1:])
    j_of = np.arange(EF) - starts[ks]
    ds = di[order]; oq = order
    rq = rk[cq[order], lid[order]]
    gq = rq // 128
    pq = rq % 128
    cs = cq[order]
    percore = []
    for c in range(NCORE):
        m_c = cs == c
        A_all = []
        P_all = []
        for g in range(NFT):
            A = np.full((JF[g], 128), NCORE * NLOC, np.int64)
            POS = np.full((JF[g], 128), -1, np.int64)
            m = m_c & (gq == g)
            A[j_of[m], pq[m]] = ds[m]
            POS[j_of[m], pq[m]] = oq[m]
            A_all.append(A)
            P_all.append(POS)
        fei = np.vstack(A_all)            # [sum(JF), 128]
        pos = np.vstack(P_all).reshape(-1)
        percore.append({"emlid": _wrap_idx(emlid[c]),
                        "fei": _wrap_idx(fei.reshape(-1)),
                        "pos": pos})
    return percore, tuple(JF)


def _host_prep(inp):
    f32 = lambda x: np.asarray(x, np.float32)
    ei12 = np.asarray(inp["ei_12"]); ei21 = np.asarray(inp["ei_21"])
    deg1 = np.bincount(np.asarray(ei21[1]), minlength=NP)[:NP]
    deg2 = np.bincount(np.asarray(ei12[1]), minlength=NP)[:NP]
    perm = {1: _perm_from_degree(deg1), 2: _perm_from_degree(deg2)}
    inv = {t: np.argsort(perm[t]) for t in (1, 2)}

    P = {}
    for t, xn, wn, bn in ((1, "x_n1", "W_in1", "b_in1"), (2, "x_n2", "W_in2", "b_in2")):
        x = np.zeros((NP, F), np.float32)
        x[:N] = f32(inp[xn])
        P[f"xT{t}"] = np.ascontiguousarray(x[inv[t]][_rinv()].T)
        P[f"Win{t}"] = f32(inp[wn])
        P[f"binc{t}"] = np.ascontiguousarray(f32(inp[bn]).reshape(HID, 1))
        P[f"binr{t}"] = f32(inp[bn]).reshape(1, HID)
    for t in (1, 2):
        rel = "12" if t == 1 else "21"
        sfx = f"n{t}"
        for l in range(L):
            bd_a = _blockdiag(f32(inp[f"a_rel_{rel}"][l]))
            bd_m = _blockdiag(f32(inp[f"m_rel_{rel}"][l]))
            scale = np.repeat(f32(inp[f"p_rel_{rel}"][l]), D) / np.sqrt(D)
            wk = (f32(inp[f"Wk_{sfx}"][l]) @ bd_a * scale[None, :])[:, _PDH]
            bk = (f32(inp[f"bk_{sfx}"][l]) @ bd_a * scale)[_PDH]
            wv = (f32(inp[f"Wv_{sfx}"][l]) @ bd_m)[:, _PDH]
            bv = (f32(inp[f"bv_{sfx}"][l]) @ bd_m)[_PDH]
            wq = f32(inp[f"Wq_{sfx}"][l])[:, _PDH]
            bq = f32(inp[f"bq_{sfx}"][l])[_PDH]
            P[f"Wtab{t}_l{l}"] = np.ascontiguousarray(
                np.concatenate([wk, wv], axis=1))                # [128, 256]
            P[f"btab{t}_l{l}"] = np.concatenate([bk, bv]).reshape(1, 2 * HID)
            P[f"Wq{t}_l{l}"] = np.ascontiguousarray(wq)
            P[f"bq{t}_l{l}"] = bq.reshape(1, HID)
            b = 1.0 / (1.0 + np.exp(-float(inp[f"skip_{sfx}"][l])))
            P[f"Wup{t}_l{l}"] = np.ascontiguousarray(b * f32(inp[f"Wa_{sfx}"][l])[_PDH, :])
            P[f"bup{t}_l{l}"] = (b * f32(inp[f"ba_{sfx}"][l])).reshape(1, HID)
            P[f"Ibl{t}_l{l}"] = ((1.0 - b) * np.eye(HID)).astype(np.float32)
    P["ident"] = np.eye(128, dtype=np.float32)
    P["ones1"] = np.ones((1, 128), np.float32)
    packs = [P.pop("Win1"), P.pop("Win2"), P.pop("ident")]
    for t in (1, 2):
        for l in range(L):
            packs.append(P.pop(f"Wtab{t}_l{l}"))
            packs.append(P.pop(f"Wq{t}_l{l}"))
    for t in (1, 2):
        for l in range(L):
            packs.append(P.pop(f"Wup{t}_l{l}"))
            packs.append(P.pop(f"Ibl{t}_l{l}"))
    P["wpack"] = np.ascontiguousarray(np.concatenate(packs, axis=1))
    P["bincp"] = np.ascontiguousarray(
        np.concatenate([P.pop("binc1"), P.pop("binc2")], axis=1))

    e12, J12 = _prep_edges(ei12, perm[1], perm[2])
    e21, J21 = _prep_edges(ei21, perm[2], perm[1])
    fin, JF = _prep_final(np.asarray(inp["edge_index"]), perm[1], perm[2])

    # per-core my-node rows: contiguous range in R (r,b,p) row order
    for c in range(NCORE):
        fin[c]["myid"] = _wrap_idx(np.arange(c * NLOC, (c + 1) * NLOC))
    return P, e12, e21, fin, tuple(J12), tuple(J21), JF


def _build(J12, J21, JF, bias_zero=False, gcap=8, scratch=16384):
    import concourse.bass as bass
    import concourse.mybir as mybir
    from concourse import bacc, tile, library_config
    from concourse.bass import broadcast_tensor_aps

    dt = mybir.dt
    AF = mybir.ActivationFunctionType
    ALU = mybir.AluOpType
    BF = dt.bfloat16
    nc = bacc.Bacc("TRN2", dynamic_dma_scratch_size=scratch)

    SJ8 = {d: sum(J) * 8 for d, J in (("12", J12), ("21", J21))}
    JL = {"12": J12, "21": J21}
    SJF = sum(JF)
    YC = SJF

    def inP(name, shape, dty=dt.float32):
        return nc.declare_dram_parameter(name, list(shape), dty, isOutput=False)

    WCOLS = 128 * 3 + 384 * L * 2 + 256 * L * 2  # same total, [k|v]+q split
    pr = {}
    for t in (1, 2):
        pr[f"xT{t}"] = inP(f"xT{t}", [128, NP])
        pr[f"binr{t}"] = inP(f"binr{t}", [1, 128])
        for l in range(L):
            for nm, sh in (("btab", [1, 256]), ("bq", [1, 128]),
                           ("bup", [1, 128])):
                pr[f"{nm}{t}_l{l}"] = inP(f"{nm}{t}_l{l}", sh)
    pr["wpack"] = inP("wpack", [128, WCOLS])
    pr["bincp"] = inP("bincp", [128, 2])
    pr["ones1"] = inP("ones1", [1, 128])
    for dname in ("12", "21"):
        pr[f"ei{dname}"] = inP(f"ei{dname}", [128, SJ8[dname]], dt.int16)
        pr[f"pc{dname}"] = inP(f"pc{dname}", [NBLK, 128])
    pr["myid"] = inP("myid", [128, NBLK * 8], dt.int16)
    pr["emlid"] = inP("emlid", [128, NFT * 8], dt.int16)
    pr["fei"] = inP("fei", [128, SJF * 8], dt.int16)
    y_out = nc.declare_dram_parameter("y", [128, YC], dt.float32, isOutput=True)

    kv = {t: nc.dram_tensor(f"kv{t}", [NP + 128, 256], BF) for t in (1, 2)}
    xt0 = {t: nc.dram_tensor(f"xt0{t}", [NP, 128], BF) for t in (1, 2)}
    HB = NLOC // 2
    agoutT = {f"{k}{h}": nc.dram_tensor(f"agoutT{k}{h}", [NCORE, 128, HB], BF,
                                        addr_space="Shared")
              for k in ("1", "2") for h in ("a", "b")}
    agout2bf = nc.dram_tensor("agout2bf", [NCORE, NLOC, 128], BF,
                              addr_space="Shared")
    emed2c = nc.dram_tensor("emed2c", [NCORE * NLOC + 128, 256], BF)
    aginTh = {f"{t}{h}": nc.dram_tensor(f"aginT{t}{h}", [128, HB], BF)
              for t in (1, 2) for h in ("a", "b")}
    agin2b = nc.dram_tensor("agin2b", [NLOC, 128], BF)
    xtloc = nc.dram_tensor("xtloc", [NLOC, 256], BF)   # my m rows, both layers

    from contextlib import ExitStack
    with tile.TileContext(nc) as tc, ExitStack() as stack:
        nc.gpsimd.load_library(library_config.mlp)
        cp = stack.enter_context(tc.tile_pool(name="const", bufs=1))
        W = {}
        # f32 weights -> bf16 SBUF copies
        with tc.tile_pool(name="wld", bufs=1) as wp:
            wf = wp.tile([128, WCOLS], dt.float32, tag="wf")
            nc.sync.dma_start(wf[:], pr["wpack"][:])
            wb = cp.tile([128, WCOLS], BF, tag="wb")
            nc.vector.tensor_copy(wb[:], wf[:])
            off = 0
            names = ["Win1", "Win2", "ident"]
            widths = [128, 128, 128]
            for t in (1, 2):
                for l in range(L):
                    names.append(f"Wtab{t}_l{l}"); widths.append(256)
                    names.append(f"Wq{t}_l{l}"); widths.append(128)
            for t in (1, 2):
                for l in range(L):
                    names.append(f"Wup{t}_l{l}"); widths.append(128)
                    names.append(f"Ibl{t}_l{l}"); widths.append(128)
            for nm, wd in zip(names, widths):
                W[nm] = wb[:, off:off + wd]
                off += wd
            bc = cp.tile([128, 2], dt.float32, tag="bincp")
            nc.sync.dma_start(bc[:], pr["bincp"][:])
            W["binc1"] = bc[:, 0:1]
            W["binc2"] = bc[:, 1:2]
            if not bias_zero:
                for k in ("ones1", "binr1", "binr2",
                          *(f"btab{t}_l{l}" for t in (1, 2) for l in range(L)),
                          *(f"bq{t}_l{l}" for t in (1, 2) for l in range(L)),
                          *(f"bup{t}_l{l}" for t in (1, 2) for l in range(L))):
                    p = pr[k]
                    tf2 = wp.tile(list(p.shape), dt.float32,
                                  tag="wf1" + str(list(p.shape)), bufs=2)
                    nc.sync.dma_start(tf2[:], p[:])
                    t_ = cp.tile(list(p.shape), BF, tag=k)
                    nc.vector.tensor_copy(t_[:], tf2[:])
                    W[k] = t_
        for dname in ("12", "21"):
            t_ = cp.tile([128, SJ8[dname]], dt.int16, tag=f"ei{dname}")
            nc.sync.dma_start(t_[:], pr[f"ei{dname}"][:])
            W[f"ei{dname}"] = t_
            t_ = cp.tile([128, NBLK], dt.float32, tag=f"pc{dname}")
            nc.sync.dma_start(t_[:], pr[f"pc{dname}"].rearrange("b p -> p b"))
            W[f"pc{dname}"] = t_
        for k, wd in (("myid", NBLK * 8), ("emlid", NFT * 8), ("fei", SJF * 8)):
            t_ = cp.tile([128, wd], dt.int16, tag=k)
            nc.sync.dma_start(t_[:], pr[k][:])
            W[k] = t_

        def tt(eng, out, a, b, op):
            a2, b2 = broadcast_tensor_aps(a, b)
            eng.tensor_tensor(out, a2, b2, op)

        def cpy(eng, dst, src):
            if eng is nc.scalar:
                eng.copy(dst, src)
            else:
                eng.tensor_copy(dst, src)

        def gat(out_t, table, idx_sb, base8, ntiles, elem):
            # dma_gather cap: gcap tiles (gcap*128 descriptors) per call
            for g0 in range(0, ntiles, gcap):
                gn = min(gcap, ntiles - g0)
                nc.gpsimd.dma_gather(
                    out_t[:, g0:g0 + gn, :], table[:, :],
                    idx_sb[:, base8 + g0 * 8:base8 + (g0 + gn) * 8],
                    gn * 128, gn * 128, elem)

        # persistent feature tiles
        xc = {t: cp.tile([128, NP], BF, tag=f"xc{t}", name=f"xc{t}") for t in (1, 2)}
        xrow = {t: cp.tile([128, NBLK, 128], BF, tag=f"xrow{t}", name=f"xrow{t}") for t in (1, 2)}
        xnew = {t: cp.tile([128, NBLK, 128], BF, tag=f"xnew{t}", name=f"xnew{t}") for t in (1, 2)}
        qmy = {t: cp.tile([128, NBLK, 128], BF, tag=f"qmy{t}", name=f"qmy{t}") for t in (1, 2)}

        # ---------- phase 0 part A: load x, project to xc ----------
        p0stack = ExitStack()
        p0 = p0stack.enter_context(tc.tile_pool(name="p0", bufs=2))
        p0b = p0stack.enter_context(tc.tile_pool(name="p0b", bufs=1))
        p0s = p0stack.enter_context(tc.tile_pool(name="p0s", bufs=3))
        p0p = p0stack.enter_context(tc.tile_pool(name="p0ps", bufs=4, space="PSUM"))
        xb = {}
        for t in (2, 1):
            xb[t] = p0b.tile([128, NP], BF, tag=f"xb{t}", name=f"xb{t}")
            for hf in range(4):
                xf = p0.tile([128, NP // 4], dt.float32, tag="xf")
                nc.sync.dma_start(xf[:], pr[f"xT{t}"][:, bass.ts(hf, NP // 4)])
                if t == 1:
                    nc.vector.tensor_copy(xb[t][:, bass.ts(hf, NP // 4)], xf[:])
                else:
                    nc.scalar.copy(xb[t][:, bass.ts(hf, NP // 4)], xf[:])
        # zero pad rows of kv + emed (kv zero row is read by the first dir)
        z = p0s.tile([128, 256], BF, tag="z")
        nc.vector.memset(z[:], 0.0)
        for t in (1, 2):
            nc.sync.dma_start(kv[t][NP:NP + 128, :], z[:])
        nc.sync.dma_start(emed2c[NCORE * NLOC:NCORE * NLOC + 128, :], z[:])

        def make_xc(t):
            # transposed projection -> xc (xT); relu split DVE/Act
            for j in range(NP // 512):
                ps = p0p.tile([128, 512], dt.float32, tag="psP")
                nc.tensor.matmul(ps[:], W[f"Win{t}"], xb[t][:, bass.ts(j, 512)],
                                 start=True, stop=True)
                if j % 2 == 0:
                    nc.vector.tensor_scalar(
                        xc[t][:, bass.ts(j, 512)], ps[:],
                        W[f"binc{t}"], 0.0, ALU.add, ALU.max)
                else:
                    nc.scalar.activation(xc[t][:, bass.ts(j, 512)], ps[:],
                                         AF.Relu, bias=W[f"binc{t}"])

        def make_xt0(t):
            # xt0 = node-major x0, built by transposing xc tiles (xc is
            # already relu'd + biased)
            with tc.tile_pool(name="p0r", bufs=3) as rp, \
                 tc.tile_pool(name="p0rps", bufs=2, space="PSUM") as rps:
                for i4 in range(NT // 4):
                    ptr = rps.tile([128, 4, 128], BF, tag="ptr")
                    for k in range(4):
                        nc.tensor.transpose(
                            ptr[:, k, :], xc[t][:, bass.ts(i4 * 4 + k, 128)],
                            W["ident"])
                    rstage = rp.tile([128, 4, 128], BF, tag="rst")
                    cpy(nc.scalar if i4 % 2 == 0 else nc.vector,
                        rstage[:], ptr[:])
                    nc.sync.dma_start(
                        xt0[t].rearrange("(a p) c -> p a c", p=128)[:, i4 * 4:(i4 + 1) * 4, :],
                        rstage[:])

        def xrow_gather(t):
            # emitted well after make_xt0 so this Pool gather never holds
            # Pool SEQ waiting for the xt0 table writes
            gat(xrow[t], xt0[t], W["myid"], 0, NBLK, 128)

        def emit_tables(t, l, psbufs=2):
            with tc.tile_pool(name="tab", bufs=2) as tp, \
                 tc.tile_pool(name="tabps", bufs=psbufs, space="PSUM") as tps:
                NH = NT // 4
                for hh in range(4):
                    kvq = tp.tile([128, NH, 256], BF, tag="kvq")
                    for i in range(NH):
                        g = hh * NH + i
                        ps = tps.tile([128, 256], dt.float32, tag="psT")
                        nc.tensor.matmul(ps[:], xc[t][:, bass.ts(g, 128)],
                                         W[f"Wtab{t}_l{l}"], start=True,
                                         stop=bias_zero)
                        if not bias_zero:
                            nc.tensor.matmul(ps[:], W["ones1"][:1, :],
                                             W[f"btab{t}_l{l}"][:1, :],
                                             start=False, stop=True)
                        cpy(nc.scalar if i % 2 else nc.vector,
                            kvq[:, i, :], ps[:])
                    nc.sync.dma_start(
                        kv[t].rearrange("(a p) c -> p a c", p=128)[:, hh * NH:(hh + 1) * NH, :],
                        kvq[:])

        def qmy_local(t, l):
            # q rows of MY nodes, straight from local node-major x rows:
            # no q table, no gather, no collective dependency
            with tc.tile_pool(name="qml", bufs=2) as qp, \
                 tc.tile_pool(name="qmlps", bufs=1, space="PSUM") as qps:
                for b in range(NBLK):
                    ptr = qps.tile([128, 128], BF, tag="qtr")
                    nc.tensor.transpose(ptr[:], xrow[t][:, b, :], W["ident"])
                    xnT = qp.tile([128, 128], BF, tag="xnT")
                    cpy(nc.scalar if b % 2 == 0 else nc.vector, xnT[:], ptr[:])
                    psQ = qps.tile([128, 128], dt.float32, tag="psQ")
                    nc.tensor.matmul(psQ[:], xnT[:], W[f"Wq{t}_l{l}"],
                                     start=True, stop=bias_zero)
                    if not bias_zero:
                        nc.tensor.matmul(psQ[:], W["ones1"][:1, :],
                                         W[f"bq{t}_l{l}"][:1, :],
                                         start=False, stop=True)
                    cpy(nc.scalar if b % 2 == 0 else nc.vector,
                        qmy[t][:, b, :], psQ[:])

        # ---------- edge phase pools (persistent across dirs/layers) ----------
        CH = 24
        estack = ExitStack()
        epools = {}

        def open_edge_pools():
            epools["eg"] = estack.enter_context(tc.tile_pool(name="eg", bufs=4))
            epools["epw"] = estack.enter_context(tc.tile_pool(name="epw", bufs=3))
            epools["ew"] = estack.enter_context(tc.tile_pool(name="ew", bufs=2))
            epools["eta"] = estack.enter_context(tc.tile_pool(name="eta", bufs=2))
            epools["eps"] = estack.enter_context(
                tc.tile_pool(name="eps", bufs=2, space="PSUM"))
            epools["epsg"] = estack.enter_context(
                tc.tile_pool(name="epsg", bufs=2, space="PSUM"))
            epools["epsu"] = estack.enter_context(
                tc.tile_pool(name="epsu", bufs=1, space="PSUM"))

        def edge_dir(dname, st, dtt, l, do_cc=True, mid=None, mid2=None,
                     early=None):
            eg = epools["eg"]; epw = epools["epw"]; ew = epools["ew"]
            eta = epools["eta"]; eps = epools["eps"]
            epsg = epools["epsg"]; epsu = epools["epsu"]
            Js = JL[dname]
            off8 = [0] * NBLK
            acc = 0
            for b in range(NBLK):
                off8[b] = acc
                acc += Js[b] * 8
            # blocks in descending-J order
            border = sorted(range(NBLK), key=lambda b: -Js[b])
            agf = eta.tile([128, NBLK, 128], dt.float32, tag="agf",
                           name=f"agf{dname}l{l}")
            rsa = eta.tile([128, NBLK, 8], dt.float32, tag="rsa",
                           name=f"rsa{dname}l{l}")
            agT = eta.tile([128, NBLK, 128], BF, tag="agT",
                           name=f"agT{dname}l{l}")

            def block_tail(b):
                # inline normalize+gelu(tanh)+update for one dst block
                gnt = ew.tile([128, 128], BF, tag="gn")
                tt(nc.vector, gnt[:].rearrange("p (d h) -> p d h", h=8),
                   agf[:, b, :].rearrange("p (d h) -> p d h", h=8),
                   rsa[:, b, :].rearrange("p (d2 h) -> p d2 h", d2=1), ALU.mult)
                t2 = ew.tile([128, 128], BF, tag="t2")
                nc.scalar.square(t2[:], gnt[:])
                u = ew.tile([128, 128], BF, tag="u")
                nc.vector.tensor_scalar(u[:], t2[:], 0.044715, 1.0,
                                        ALU.mult, ALU.add)
                v = ew.tile([128, 128], BF, tag="v")
                tt(nc.vector, v[:], gnt[:], u[:], ALU.mult)
                th = ew.tile([128, 128], BF, tag="th")
                nc.scalar.activation(th[:], v[:], AF.Tanh,
                                     scale=0.7978845608)
                hm = ew.tile([128, 128], BF, tag="hm")
                nc.vector.tensor_scalar(hm[:], th[:], 0.5, 0.5,
                                        ALU.mult, ALU.add)
                gb = ew.tile([128, 128], BF, tag="gb")
                tt(nc.vector, gb[:], gnt[:], hm[:], ALU.mult)
                trp = epsu.tile([128, 128], BF, tag="trp")
                nc.tensor.transpose(trp[:], gb[:], W["ident"])
                gT = ew.tile([128, 128], BF, tag="gT")
                nc.scalar.copy(gT[:], trp[:])
                psU = epsu.tile([128, 128], dt.float32, tag="psU")
                nc.tensor.matmul(psU[:], gT[:], W[f"Wup{dtt}_l{l}"],
                                 start=True, stop=False)
                if not bias_zero:
                    nc.tensor.matmul(psU[:], W["ones1"][:1, :],
                                     W[f"bup{dtt}_l{l}"][:1, :],
                                     start=False, stop=False)
                nc.tensor.matmul(psU[:], W[f"Ibl{dtt}_l{l}"],
                                 xrow[dtt][:, b, :], start=False, stop=True)
                nc.scalar.copy(xnew[dtt][:, b, :], psU[:])
                if do_cc and l == 0:
                    trpn = epsu.tile([128, 128], BF, tag="trp")
                    nc.tensor.transpose(trpn[:], xnew[dtt][:, b, :],
                                        W["ident"])
                    cpy(nc.scalar if b % 2 == 0 else nc.vector,
                        agT[:, b, :], trpn[:])

            def write_agin(b0, b1):
                # stage this half's cc input early; the collective itself is
                # emitted later so it never holds Pool SEQ waiting for input
                if not do_cc:
                    return
                if l == 0:
                    h = "a" if b0 else "b"
                    nc.sync.dma_start(aginTh[f"{dtt}{h}"][:, :],
                                      agT[:, b0:b1, :].rearrange(
                                          "p b c -> p (b c)"))
                else:
                    nc.sync.dma_start(
                        agin2b.rearrange("(b p) c -> p b c", p=128)[:, b0:b1, :],
                        xnew[dtt][:, b0:b1, :])

            def cc_only(h, b0, b1):
                if not do_cc:
                    return
                if l == 0:
                    nc.gpsimd.collective_compute(
                        "AllGather", mybir.AluOpType.bypass,
                        ins=[aginTh[f"{dtt}{h}"][:, :]],
                        outs=[agoutT[f"{dtt}{h}"][:]],
                        replica_groups=[list(range(NCORE))])
                elif h == "b":
                    # layer 1: one full collective (gates the final phase;
                    # splitting it only adds fixed overhead)
                    nc.gpsimd.collective_compute(
                        "AllGather", mybir.AluOpType.bypass,
                        ins=[agin2b[:]], outs=[agout2bf[:]],
                        replica_groups=[list(range(NCORE))])

            done = 0
            for b in border:
                J = Js[b]
                nch = (J + CH - 1) // CH
                psG = sacc = None
                for ci in range(nch):
                    j0 = ci * CH
                    jn = min(CH, J - j0)
                    first = ci == 0
                    last = ci == nch - 1
                    coff8 = off8[b] + j0 * 8
                    kvg = eg.tile([128, CH, 256], BF, tag="kvg")
                    gat(kvg, kv[st], W[f"ei{dname}"], coff8, jn, 256)
                    prod = epw.tile([128, CH, 128], BF, tag="pw", name="prod")
                    tt(nc.vector, prod[:, 0:jn, :], kvg[:, 0:jn, 0:128],
                       qmy[dtt][:, b:b + 1, :], ALU.mult)
                    # alpha[p, j, h] = sum_d prod[p, j, d*8+h]: PE
                    # identity-matmul accumulation over the 16 d-slabs
                    psA = eps.tile([128, CH * 8], dt.float32, tag="psA")
                    for dd in range(D):
                        nc.tensor.matmul(psA[:, 0:jn * 8], W["ident"],
                                         prod[:, 0:jn, bass.ts(dd, 8)],
                                         start=(dd == 0), stop=(dd == D - 1))
                    eB = ew.tile([128, CH, 8], BF, tag="eB")
                    nc.scalar.activation(
                        eB[:, 0:jn, :],
                        psA[:, 0:jn * 8].rearrange("p (j h) -> p j h", h=8),
                        AF.Exp)
                    # wv[p, j, d*8+h] = v * e  (2x: d-major v, h innermost)
                    wv = epw.tile([128, CH, 128], BF, tag="pw", name="wv")
                    tt(nc.vector,
                       wv[:, 0:jn, :].rearrange("p j (d h) -> p j d h", h=8),
                       kvg[:, 0:jn, 128:256].rearrange("p j (d h) -> p j d h", h=8),
                       eB[:, 0:jn, :].rearrange("p j (d2 h) -> p j d2 h", d2=1),
                       ALU.mult)
                    # s tree over j (in-place on eB), bf16 accum
                    with nc.allow_low_precision(reason="softmax denom bf16 tree"):
                        Jc = jn
                        while Jc > 1:
                            h1 = (Jc + 1) // 2
                            tt(nc.vector, eB[:, 0:Jc - h1, :], eB[:, 0:Jc - h1, :],
                               eB[:, h1:Jc, :], ALU.add)
                            Jc = h1
                    if first:
                        sacc = ew.tile([128, 8], dt.float32, tag="sacc")
                        nc.vector.tensor_copy(sacc[:], eB[:, 0, :])
                    else:
                        tt(nc.vector, sacc[:], sacc[:], eB[:, 0, :], ALU.add)
                    # agg[p, dh] += sum_j wv: 4-tile-packed identity matmuls
                    if first:
                        psG = epsg.tile([128, 4, 128], dt.float32, tag="psG")
                    nst = (jn + 3) // 4
                    for g in range(nst):
                        gw = min(4, jn - g * 4)
                        nc.tensor.matmul(psG[:, 0:gw, :], W["ident"],
                                         wv[:, g * 4:g * 4 + gw, :],
                                         start=(first and g == 0),
                                         stop=(last and g == nst - 1))
                    if not last:
                        continue
                    sden = ew.tile([128, 8], dt.float32, tag="sden")
                    tt(nc.vector, sden[:, :], sacc[:],
                       W[f"pc{dname}"][:, b:b + 1], ALU.subtract)
                    nc.vector.reciprocal(rsa[:, b, :], sden[:])
                    nc.vector.tensor_reduce(
                        agf[:, b, :], psG[:].rearrange("p r c -> p c r"),
                        mybir.AxisListType.X, ALU.add)
                block_tail(b)
                done += 1
                if done == 2 and early is not None:
                    early()
                if done == NBLK // 2:
                    write_agin(NBLK // 2, NBLK)
                    if mid is not None:
                        mid()
                if done == NBLK // 2 + 2 and l == 0:
                    # input staged 2 blocks ago -> no Pool SEQ hold here
                    cc_only("a", NBLK // 2, NBLK)
                if done == NBLK - 1 and mid2 is not None:
                    mid2()
            write_agin(0, NBLK // 2)
            if dtt == 1:
                # my updated type-1 rows into the local final table
                nc.sync.dma_start(
                    xtloc.rearrange("(b p) c -> p b c", p=128)[:, :, l * 128:(l + 1) * 128],
                    xnew[1][:])
            return (lambda: cc_only("b", 0, NBLK // 2)) if do_cc else None

        def post_xc(t):
            # xc[t] column order IS the AllGather row order, so the rebuild
            # is 16 plain contiguous reads straight into the xc tile
            for h, b0 in (("a", NBLK // 2), ("b", 0)):
                for r in range(NCORE):
                    nc.sync.dma_start(
                        xc[t][:, r * NLOC + b0 * 128:
                              r * NLOC + b0 * 128 + HB],
                        agoutT[f"{t}{h}"][r])

        def emed_l0_from_xc():
            # final ed table's layer-0 columns = node-major transposes of
            # xc2 (runs in layer-1 slack, off every critical path)
            with tc.tile_pool(name="eml0", bufs=3) as pp, \
                 tc.tile_pool(name="eml0ps", bufs=2, space="PSUM") as ppp:
                emv = emed2c[0:NCORE * NLOC, 0:128].rearrange(
                    "(g p) c -> p g c", p=128)
                for i4 in range(NT // 4):
                    ptr = ppp.tile([128, 4, 128], BF, tag="ptr")
                    for k in range(4):
                        nc.tensor.transpose(
                            ptr[:, k, :],
                            xc[2][:, bass.ts(i4 * 4 + k, 128)], W["ident"])
                    rstage = pp.tile([128, 4, 128], BF, tag="rst")
                    cpy(nc.scalar if i4 % 2 == 0 else nc.vector,
                        rstage[:], ptr[:])
                    nc.sync.dma_start(emv[:, i4 * 4:(i4 + 1) * 4, :],
                                      rstage[:])

        def copy_l1_half():
            # stage agout2bf into the final ed table's layer-1 columns
            with tc.tile_pool(name="cl1", bufs=3) as pp:
                emv = emed2c[0:NCORE * NLOC, 128:256].rearrange(
                    "(g p) c -> p g c", p=128)
                srcv = agout2bf[:].rearrange("r (b p) c -> p (r b) c",
                                             p=128)
                for q4 in range(NT // 4):
                    rd = pp.tile([128, 4, 128], BF, tag="rd")
                    nc.sync.dma_start(rd[:], srcv[:, q4 * 4:(q4 + 1) * 4, :])
                    nc.sync.dma_start(emv[:, q4 * 4:(q4 + 1) * 4, :], rd[:])

        # ---------- layers: phase stamps act as scheduler barriers; keep
        # one only where collective-gated work could otherwise poison an
        # independent engine stream ----------
        tc.tile_set_cur_wait(1)
        make_xc(2)
        emit_tables(2, 0, psbufs=4)
        make_xc(1)
        make_xt0(1)
        xrow_gather(1)
        qmy_local(1, 0)
        p0stack.close()
        open_edge_pools()
        tc.tile_set_cur_wait(2)
        emit_tables(1, 0)
        cc1b = edge_dir("21", 2, 1, 0,
                        mid=lambda: make_xt0(2),
                        mid2=lambda: (xrow_gather(2), qmy_local(2, 0)))
        cc2b = edge_dir("12", 1, 2, 0, early=cc1b)
        xrow, xnew = xnew, xrow
        tc.tile_set_cur_wait(3)
        cc2b()
        post_xc(1)
        emit_tables(1, 1)
        qmy_local(2, 1)
        qmy_local(1, 1)
        cc2bb = edge_dir("12", 1, 2, 1)
        tc.tile_set_cur_wait(4)
        cc2bb()
        post_xc(2)
        emit_tables(2, 1)
        emed_l0_from_xc()
        edge_dir("21", 2, 1, 1, do_cc=False)
        copy_l1_half()
        tc.tile_set_cur_wait(5)
        estack.close()

        # ---------- final gather-dot (m-grouped, local dense Em side) ----------
        with tc.tile_pool(name="fin", bufs=4) as fp, \
             tc.tile_pool(name="fpb", bufs=3) as fpb, \
             tc.tile_pool(name="fem", bufs=1) as fem, \
             tc.tile_pool(name="finps", bufs=4, space="PSUM") as fps, \
             tc.tile_pool(name="ybuf", bufs=1) as yp:
            ysb = yp.tile([128, YC], dt.float32, tag="ysb")
            emT = fem.tile([128, NFT, 256], BF, tag="emT")
            gat(emT, xtloc, W["emlid"], 0, NFT, 256)
            col = 0
            for g in range(NFT):
                base8 = sum(JF[:g]) * 8
                for j0 in range(0, JF[g], 8):
                    gn_t = min(8, JF[g] - j0)
                    ed = fp.tile([128, 8, 256], BF, tag="ed")
                    gat(ed, emed2c, W["fei"], base8 + j0 * 8, gn_t, 256)
                    pb = fpb.tile([128, 8, 256], BF, tag="pb")
                    tt(nc.vector, pb[:, 0:gn_t, :], ed[:, 0:gn_t, :],
                       emT[:, g:g + 1, :], ALU.mult)
                    # slab-sum on PE: psY[p, t, i] = sum_s pb[p, t, s*16+i]
                    psY = fps.tile([128, 8, 16], dt.float32, tag="psY")
                    for s in range(16):
                        nc.tensor.matmul(psY[:, 0:gn_t, :], W["ident"],
                                         pb[:, 0:gn_t, bass.ts(s, 16)],
                                         start=(s == 0), stop=(s == 15))
                    nc.vector.tensor_reduce(
                        ysb[:, col:col + gn_t], psY[:, 0:gn_t, :],
                        mybir.AxisListType.X, ALU.add)
                    col += gn_t
            nc.sync.dma_start(y_out[:, :], ysb[:])
    nc.compile()
    return nc


_CACHE = {}
_last_key = None


def kernel(**inputs):
    global _last_key
    from concourse.bass_utils import run_bass_kernel_spmd
    P, e12, e21, fin, J12, J21, JF = _host_prep(inputs)
    bz = all(not np.any(np.asarray(inputs[k]))
             for k in inputs if k.startswith("b"))
    key = (J12, J21, JF, bz)
    _last_key = key
    if key not in _CACHE:
        _CACHE[key] = _build(J12, J21, JF, bias_zero=bz)
    nc = _CACHE[key]
    in_maps = []
    for c in range(NCORE):
        m = dict(P)
        m["ei12"] = e12[c]["idx"]; m["pc12"] = e12[c]["padc"]
        m["ei21"] = e21[c]["idx"]; m["pc21"] = e21[c]["padc"]
        m["myid"] = fin[c]["myid"]
        m["emlid"] = fin[c]["emlid"]; m["fei"] = fin[c]["fei"]
        in_maps.append(m)
    res = run_bass_kernel_spmd(nc, in_maps, list(range(NCORE)))
    y = np.zeros((EF,), np.float32)
    for c in range(NCORE):
        yc = np.asarray(res.results[c]["y"])      # [128, YC]
        ylin = yc.T.ravel()                        # slot (col, p) order
        pos = fin[c]["pos"]
        mreal = pos >= 0
        y[pos[mreal]] = ylin[mreal]
    return y.reshape(EF, 1)


# revision 33
# speedup vs baseline: 1.0185x; 1.0185x over previous
"""Trainium2 Bass kernel for 2-layer HGT message passing + sparse gather-dot,
sharded over 8 NeuronCores.

Layout strategy (v3):
 - Nodes of each type are RELABELED host-side by in-degree rank:
   new_id = band*128 + slot, band = rank//128 (80 bands, degree-sorted),
   core(band) = band % 8.  All indices (edges, final queries) are remapped
   through the permutation, so the device never sees it.
 - Edge phase uses a dst-per-partition layout: for a 128-dst block, slot
   (p, j) holds the j-th in-edge of dst p.  J_b = max in-block degree is a
   compile-time constant per block.  Blocks run in descending-J order so
   the heavy block's gathers prefetch first and the pipeline tail is the
   smallest block.
 - gelu+update tails are deferred to the end of each direction so the
   Activation engine swaps tables (Exp<->Gelu) only twice per direction.
 - The per-type AllGather writes DIRECTLY into the emed staging table via
   a strided output AP (no stage readback / emed rewrite).
 - Final gather-dot: queries are grouped by their m-node's OWNER core and
   packed into query-count-sorted 128-node tiles (slot (p, j) = j-th query
   of m-node p).  The dense Em side comes from a core-LOCAL table (both
   layers' outputs, written without any collective), so the layer-1 type-1
   AllGather is not needed at all (3 collectives, not 4).  Only Ed rows
   are gathered per query (from emed2, gated on the EARLY layer-1
   collective) -> half the final gather traffic of the pair-gather scheme
   and no collective on the final critical path.
 - All tables and gathered data are bf16 (512B gather rows).  PSUM stays
   f32.
"""
import numpy as np

N = 10000
NP = 10240          # padded node count (80 tiles of 128)
NT = NP // 128      # 80 tiles
NCORE = 8
NBLK = NT // NCORE  # 10 blocks (dst tiles) per core
NLOC = NBLK * 128   # 1280 nodes owned per core
F = 128; HID = 128; H = 8; D = 16; L = 2
EF = 500000
NFT = NBLK          # 10 final m-tiles per core
ZROW = NP           # zero row in kv/emed tables used by padding slots


def _wrap_idx(idx):
    """int index list (len%16==0) -> [128, len//16] int16 in gather format."""
    a = np.asarray(idx, np.int16).reshape(-1, 16).T
    return np.ascontiguousarray(np.tile(a, (8, 1)))


def _blockdiag(a):
    out = np.zeros((HID, HID), np.float32)
    for h in range(H):
        out[h * D:(h + 1) * D, h * D:(h + 1) * D] = a[h]
    return out


# column permutation (h,d) -> d-major (d*8+h)
_PDH = np.zeros(HID, np.int64)
for _h in range(H):
    for _d in range(D):
        _PDH[_d * H + _h] = _h * D + _d   # new col i=d*8+h takes old col h*16+d


def _perm_from_degree(deg):
    """deg[NP] -> perm (old->new), degree-ascending bands dealt round-robin."""
    order = np.argsort(deg, kind="stable")       # order[r] = old id
    perm = np.empty(NP, np.int64)
    r = np.arange(NP)
    perm[order] = r                               # new_id = rank
    return perm


def _rmap(x):
    """permuted id -> table row in AllGather output order (r, b, p)."""
    band = x // 128
    return ((band % NCORE) * NBLK + band // NCORE) * 128 + x % 128


_RINV = None


def _rinv():
    global _RINV
    if _RINV is None:
        inv = np.empty(NP, np.int64)
        inv[_rmap(np.arange(NP))] = np.arange(NP)
        _RINV = inv
    return _RINV


def _prep_edges(ei, perm_s, perm_d):
    """-> per-core dict(idx [128, SJ*8] i16, padc [NBLK,128] f32), J list."""
    s = _rmap(perm_s[np.asarray(ei[0])])
    d = perm_d[np.asarray(ei[1])]
    band = d // 128
    core = band % NCORE
    blk = band // NCORE
    p = d % 128
    # j-th edge of each dst: stable sort by d, position within group
    order = np.argsort(d, kind="stable")
    ds = d[order]
    cnt = np.bincount(d, minlength=NP)
    starts = np.zeros(NP + 1, np.int64)
    np.cumsum(cnt, out=starts[1:])
    j_of = np.arange(len(ds)) - starts[ds]
    # J per (core, blk): max degree in band
    J = np.zeros((NCORE, NBLK), np.int64)
    for b in range(NT):
        mx = cnt[b * 128:(b + 1) * 128].max()
        J[b % NCORE, b // NCORE] = max(J[b % NCORE, b // NCORE], mx)
    Jb = [max(1, int(J[:, b].max())) for b in range(NBLK)]  # same for all cores
    out = []
    ss = s[order]
    cs = core[order]; bs = blk[order]; ps = p[order]
    for c in range(NCORE):
        idxs = []
        padc = np.zeros((NBLK, 128), np.float32)
        m_c = cs == c
        for b in range(NBLK):
            Jcb = Jb[b]
            A = np.full((Jcb, 128), ZROW, np.int64)
            m = m_c & (bs == b)
            A[j_of[m], ps[m]] = ss[m]
            band_cnt = cnt[(b * NCORE + c) * 128:(b * NCORE + c + 1) * 128]
            # 1e-3 denominator bias keeps zero-degree rows finite (0*1000=0);
            # relative effect on real weights ~1e-3/32, far under tolerance
            padc[b, :] = (Jcb - band_cnt).astype(np.float32) - 1e-3
            idxs.append(_wrap_idx(A.reshape(-1)))
        out.append({"idx": np.ascontiguousarray(np.hstack(idxs)),
                    "padc": padc})
    return out, Jb


def _prep_final(eidx, perm1, perm2):
    """Queries grouped by m-node owner core; count-sorted local tiles.

    Returns per-core dicts (emlid: local m ids per tile for the dense Em
    gather, fei: ed gather idx per slot, pos: slot -> original query id or
    -1) and JF (per-tile max query count, shared across cores).
    """
    mi = perm1[np.asarray(eidx[0])]
    di = perm2[np.asarray(eidx[1])]
    band = mi // 128
    cq = band % NCORE
    lid = (band // NCORE) * 128 + mi % 128       # local row in xtloc
    # ed side reads the emed2c table whose rows are in AllGather output
    # order (r, b, p)
    di = _rmap(di)
    key = cq * NLOC + lid
    cntq = np.bincount(key, minlength=NCORE * NLOC).reshape(NCORE, NLOC)
    rk = np.empty((NCORE, NLOC), np.int64)
    sorted_cnt = np.empty((NCORE, NLOC), np.int64)
    emlid = np.empty((NCORE, NLOC), np.int64)
    for c in range(NCORE):
        o = np.argsort(-cntq[c], kind="stable")
        rk[c, o] = np.arange(NLOC)
        sorted_cnt[c] = cntq[c][o]
        emlid[c] = o                              # rank -> local id
    JF = [max(1, int(sorted_cnt[:, g * 128].max())) for g in range(NFT)]
    # position of each query within its (core, m-node) group
    order = np.argsort(key, kind="stable")
    ks = key[order]
    starts = np.zeros(NCORE * NLOC + 1, np.int64)
    np.cumsum(cntq.reshape(-1), out=starts[1:])
    j_of = np.arange(EF) - starts[ks]
    ds = di[order]; oq = order
    rq = rk[cq[order], lid[order]]
    gq = rq // 128
    pq = rq % 128
    cs = cq[order]
    percore = []
    for c in range(NCORE):
        m_c = cs == c
        A_all = []
        P_all = []
        for g in range(NFT):
            A = np.full((JF[g], 128), NCORE * NLOC, np.int64)
            POS = np.full((JF[g], 128), -1, np.int64)
            m = m_c & (gq == g)
            A[j_of[m], pq[m]] = ds[m]
            POS[j_of[m], pq[m]] = oq[m]
            A_all.append(A)
            P_all.append(POS)
        fei = np.vstack(A_all)            # [sum(JF), 128]
        pos = np.vstack(P_all).reshape(-1)
        percore.append({"emlid": _wrap_idx(emlid[c]),
                        "fei": _wrap_idx(fei.reshape(-1)),
                        "pos": pos})
    return percore, tuple(JF)


def _host_prep(inp):
    f32 = lambda x: np.asarray(x, np.float32)
    ei12 = np.asarray(inp["ei_12"]); ei21 = np.asarray(inp["ei_21"])
    deg1 = np.bincount(np.asarray(ei21[1]), minlength=NP)[:NP]
    deg2 = np.bincount(np.asarray(ei12[1]), minlength=NP)[:NP]
    perm = {1: _perm_from_degree(deg1), 2: _perm_from_degree(deg2)}
    inv = {t: np.argsort(perm[t]) for t in (1, 2)}

    P = {}
    for t, xn, wn, bn in ((1, "x_n1", "W_in1", "b_in1"), (2, "x_n2", "W_in2", "b_in2")):
        x = np.zeros((NP, F), np.float32)
        x[:N] = f32(inp[xn])
        import ml_dtypes
        P[f"xT{t}"] = np.ascontiguousarray(
            x[inv[t]][_rinv()].T.astype(ml_dtypes.bfloat16))
        P[f"Win{t}"] = f32(inp[wn])
        P[f"binc{t}"] = np.ascontiguousarray(f32(inp[bn]).reshape(HID, 1))
        P[f"binr{t}"] = f32(inp[bn]).reshape(1, HID)
    for t in (1, 2):
        rel = "12" if t == 1 else "21"
        sfx = f"n{t}"
        for l in range(L):
            bd_a = _blockdiag(f32(inp[f"a_rel_{rel}"][l]))
            bd_m = _blockdiag(f32(inp[f"m_rel_{rel}"][l]))
            scale = np.repeat(f32(inp[f"p_rel_{rel}"][l]), D) / np.sqrt(D)
            wk = (f32(inp[f"Wk_{sfx}"][l]) @ bd_a * scale[None, :])[:, _PDH]
            bk = (f32(inp[f"bk_{sfx}"][l]) @ bd_a * scale)[_PDH]
            wv = (f32(inp[f"Wv_{sfx}"][l]) @ bd_m)[:, _PDH]
            bv = (f32(inp[f"bv_{sfx}"][l]) @ bd_m)[_PDH]
            wq = f32(inp[f"Wq_{sfx}"][l])[:, _PDH]
            bq = f32(inp[f"bq_{sfx}"][l])[_PDH]
            P[f"Wtab{t}_l{l}"] = np.ascontiguousarray(
                np.concatenate([wk, wv], axis=1))                # [128, 256]
            P[f"btab{t}_l{l}"] = np.concatenate([bk, bv]).reshape(1, 2 * HID)
            P[f"Wq{t}_l{l}"] = np.ascontiguousarray(wq)
            P[f"bq{t}_l{l}"] = bq.reshape(1, HID)
            b = 1.0 / (1.0 + np.exp(-float(inp[f"skip_{sfx}"][l])))
            P[f"Wup{t}_l{l}"] = np.ascontiguousarray(b * f32(inp[f"Wa_{sfx}"][l])[_PDH, :])
            P[f"bup{t}_l{l}"] = (b * f32(inp[f"ba_{sfx}"][l])).reshape(1, HID)
            P[f"Ibl{t}_l{l}"] = ((1.0 - b) * np.eye(HID)).astype(np.float32)
    P["ident"] = np.eye(128, dtype=np.float32)
    P["ones1"] = np.ones((1, 128), np.float32)
    packs = [P.pop("Win1"), P.pop("Win2"), P.pop("ident")]
    for t in (1, 2):
        for l in range(L):
            packs.append(P.pop(f"Wtab{t}_l{l}"))
            packs.append(P.pop(f"Wq{t}_l{l}"))
    for t in (1, 2):
        for l in range(L):
            packs.append(P.pop(f"Wup{t}_l{l}"))
            packs.append(P.pop(f"Ibl{t}_l{l}"))
    P["wpack"] = np.ascontiguousarray(np.concatenate(packs, axis=1))
    P["bincp"] = np.ascontiguousarray(
        np.concatenate([P.pop("binc1"), P.pop("binc2")], axis=1))

    e12, J12 = _prep_edges(ei12, perm[1], perm[2])
    e21, J21 = _prep_edges(ei21, perm[2], perm[1])
    fin, JF = _prep_final(np.asarray(inp["edge_index"]), perm[1], perm[2])

    # per-core my-node rows: contiguous range in R (r,b,p) row order
    for c in range(NCORE):
        fin[c]["myid"] = _wrap_idx(np.arange(c * NLOC, (c + 1) * NLOC))
    return P, e12, e21, fin, tuple(J12), tuple(J21), JF


def _build(J12, J21, JF, bias_zero=False, gcap=8, scratch=16384):
    import concourse.bass as bass
    import concourse.mybir as mybir
    from concourse import bacc, tile, library_config
    from concourse.bass import broadcast_tensor_aps

    dt = mybir.dt
    AF = mybir.ActivationFunctionType
    ALU = mybir.AluOpType
    BF = dt.bfloat16
    nc = bacc.Bacc("TRN2", dynamic_dma_scratch_size=scratch)

    SJ8 = {d: sum(J) * 8 for d, J in (("12", J12), ("21", J21))}
    JL = {"12": J12, "21": J21}
    SJF = sum(JF)
    YC = SJF

    def inP(name, shape, dty=dt.float32):
        return nc.declare_dram_parameter(name, list(shape), dty, isOutput=False)

    WCOLS = 128 * 3 + 384 * L * 2 + 256 * L * 2  # same total, [k|v]+q split
    pr = {}
    for t in (1, 2):
        pr[f"xT{t}"] = inP(f"xT{t}", [128, NP], BF)
        pr[f"binr{t}"] = inP(f"binr{t}", [1, 128])
        for l in range(L):
            for nm, sh in (("btab", [1, 256]), ("bq", [1, 128]),
                           ("bup", [1, 128])):
                pr[f"{nm}{t}_l{l}"] = inP(f"{nm}{t}_l{l}", sh)
    pr["wpack"] = inP("wpack", [128, WCOLS])
    pr["bincp"] = inP("bincp", [128, 2])
    pr["ones1"] = inP("ones1", [1, 128])
    for dname in ("12", "21"):
        pr[f"ei{dname}"] = inP(f"ei{dname}", [128, SJ8[dname]], dt.int16)
        pr[f"pc{dname}"] = inP(f"pc{dname}", [NBLK, 128])
    pr["myid"] = inP("myid", [128, NBLK * 8], dt.int16)
    pr["emlid"] = inP("emlid", [128, NFT * 8], dt.int16)
    pr["fei"] = inP("fei", [128, SJF * 8], dt.int16)
    y_out = nc.declare_dram_parameter("y", [128, YC], dt.float32, isOutput=True)

    kv = {t: nc.dram_tensor(f"kv{t}", [NP + 128, 256], BF) for t in (1, 2)}
    xt0 = {t: nc.dram_tensor(f"xt0{t}", [NP, 128], BF) for t in (1, 2)}
    HB = NLOC // 2
    agoutT = {f"{k}{h}": nc.dram_tensor(f"agoutT{k}{h}", [NCORE, 128, HB], BF,
                                        addr_space="Shared")
              for k in ("1", "2") for h in ("a", "b")}
    agout2bf = nc.dram_tensor("agout2bf", [NCORE, NLOC, 128], BF,
                              addr_space="Shared")
    emed2c = nc.dram_tensor("emed2c", [NCORE * NLOC + 128, 256], BF)
    aginTh = {f"{t}{h}": nc.dram_tensor(f"aginT{t}{h}", [128, HB], BF)
              for t in (1, 2) for h in ("a", "b")}
    agin2b = nc.dram_tensor("agin2b", [NLOC, 128], BF)
    xtloc = nc.dram_tensor("xtloc", [NLOC, 256], BF)   # my m rows, both layers

    from contextlib import ExitStack
    with tile.TileContext(nc) as tc, ExitStack() as stack:
        nc.gpsimd.load_library(library_config.mlp)
        cp = stack.enter_context(tc.tile_pool(name="const", bufs=1))
        W = {}
        # f32 weights -> bf16 SBUF copies
        with tc.tile_pool(name="wld", bufs=1) as wp:
            wf = wp.tile([128, WCOLS], dt.float32, tag="wf")
            nc.sync.dma_start(wf[:], pr["wpack"][:])
            wb = cp.tile([128, WCOLS], BF, tag="wb")
            nc.vector.tensor_copy(wb[:], wf[:])
            off = 0
            names = ["Win1", "Win2", "ident"]
            widths = [128, 128, 128]
            for t in (1, 2):
                for l in range(L):
                    names.append(f"Wtab{t}_l{l}"); widths.append(256)
                    names.append(f"Wq{t}_l{l}"); widths.append(128)
            for t in (1, 2):
                for l in range(L):
                    names.append(f"Wup{t}_l{l}"); widths.append(128)
                    names.append(f"Ibl{t}_l{l}"); widths.append(128)
            for nm, wd in zip(names, widths):
                W[nm] = wb[:, off:off + wd]
                off += wd
            bc = cp.tile([128, 2], dt.float32, tag="bincp")
            nc.sync.dma_start(bc[:], pr["bincp"][:])
            W["binc1"] = bc[:, 0:1]
            W["binc2"] = bc[:, 1:2]
            if not bias_zero:
                for k in ("ones1", "binr1", "binr2",
                          *(f"btab{t}_l{l}" for t in (1, 2) for l in range(L)),
                          *(f"bq{t}_l{l}" for t in (1, 2) for l in range(L)),
                          *(f"bup{t}_l{l}" for t in (1, 2) for l in range(L))):
                    p = pr[k]
                    tf2 = wp.tile(list(p.shape), dt.float32,
                                  tag="wf1" + str(list(p.shape)), bufs=2)
                    nc.sync.dma_start(tf2[:], p[:])
                    t_ = cp.tile(list(p.shape), BF, tag=k)
                    nc.vector.tensor_copy(t_[:], tf2[:])
                    W[k] = t_
        for dname in ("12", "21"):
            t_ = cp.tile([128, SJ8[dname]], dt.int16, tag=f"ei{dname}")
            nc.sync.dma_start(t_[:], pr[f"ei{dname}"][:])
            W[f"ei{dname}"] = t_
            t_ = cp.tile([128, NBLK], dt.float32, tag=f"pc{dname}")
            nc.sync.dma_start(t_[:], pr[f"pc{dname}"].rearrange("b p -> p b"))
            W[f"pc{dname}"] = t_
        for k, wd in (("myid", NBLK * 8), ("emlid", NFT * 8), ("fei", SJF * 8)):
            t_ = cp.tile([128, wd], dt.int16, tag=k)
            nc.sync.dma_start(t_[:], pr[k][:])
            W[k] = t_

        def tt(eng, out, a, b, op):
            a2, b2 = broadcast_tensor_aps(a, b)
            eng.tensor_tensor(out, a2, b2, op)

        def cpy(eng, dst, src):
            if eng is nc.scalar:
                eng.copy(dst, src)
            else:
                eng.tensor_copy(dst, src)

        def gat(out_t, table, idx_sb, base8, ntiles, elem):
            # dma_gather cap: gcap tiles (gcap*128 descriptors) per call
            for g0 in range(0, ntiles, gcap):
                gn = min(gcap, ntiles - g0)
                nc.gpsimd.dma_gather(
                    out_t[:, g0:g0 + gn, :], table[:, :],
                    idx_sb[:, base8 + g0 * 8:base8 + (g0 + gn) * 8],
                    gn * 128, gn * 128, elem)

        # persistent feature tiles
        xc = {t: cp.tile([128, NP], BF, tag=f"xc{t}", name=f"xc{t}") for t in (1, 2)}
        xrow = {t: cp.tile([128, NBLK, 128], BF, tag=f"xrow{t}", name=f"xrow{t}") for t in (1, 2)}
        xnew = {t: cp.tile([128, NBLK, 128], BF, tag=f"xnew{t}", name=f"xnew{t}") for t in (1, 2)}
        qmy = {t: cp.tile([128, NBLK, 128], BF, tag=f"qmy{t}", name=f"qmy{t}") for t in (1, 2)}

        # ---------- phase 0 part A: load x, project to xc ----------
        p0stack = ExitStack()
        p0 = p0stack.enter_context(tc.tile_pool(name="p0", bufs=2))
        p0b = p0stack.enter_context(tc.tile_pool(name="p0b", bufs=1))
        p0s = p0stack.enter_context(tc.tile_pool(name="p0s", bufs=3))
        p0p = p0stack.enter_context(tc.tile_pool(name="p0ps", bufs=4, space="PSUM"))
        xb = {}
        for t in (2, 1):
            xb[t] = p0b.tile([128, NP], BF, tag=f"xb{t}", name=f"xb{t}")
            for hf in range(4):
                nc.sync.dma_start(xb[t][:, bass.ts(hf, NP // 4)],
                                  pr[f"xT{t}"][:, bass.ts(hf, NP // 4)])
        # zero pad rows of kv + emed (kv zero row is read by the first dir)
        z = p0s.tile([128, 256], BF, tag="z")
        nc.vector.memset(z[:], 0.0)
        for t in (1, 2):
            nc.sync.dma_start(kv[t][NP:NP + 128, :], z[:])
        nc.sync.dma_start(emed2c[NCORE * NLOC:NCORE * NLOC + 128, :], z[:])

        def make_xc(t):
            # transposed projection -> xc (xT); relu split DVE/Act
            for j in range(NP // 512):
                ps = p0p.tile([128, 512], dt.float32, tag="psP")
                nc.tensor.matmul(ps[:], W[f"Win{t}"], xb[t][:, bass.ts(j, 512)],
                                 start=True, stop=True)
                if j % 2 == 0:
                    nc.vector.tensor_scalar(
                        xc[t][:, bass.ts(j, 512)], ps[:],
                        W[f"binc{t}"], 0.0, ALU.add, ALU.max)
                else:
                    nc.scalar.activation(xc[t][:, bass.ts(j, 512)], ps[:],
                                         AF.Relu, bias=W[f"binc{t}"])

        def make_xt0(t):
            # xt0 = node-major x0, built by transposing xc tiles (xc is
            # already relu'd + biased)
            with tc.tile_pool(name="p0r", bufs=3) as rp, \
                 tc.tile_pool(name="p0rps", bufs=2, space="PSUM") as rps:
                for i4 in range(NT // 4):
                    ptr = rps.tile([128, 4, 128], BF, tag="ptr")
                    for k in range(4):
                        nc.tensor.transpose(
                            ptr[:, k, :], xc[t][:, bass.ts(i4 * 4 + k, 128)],
                            W["ident"])
                    rstage = rp.tile([128, 4, 128], BF, tag="rst")
                    cpy(nc.scalar if i4 % 2 == 0 else nc.vector,
                        rstage[:], ptr[:])
                    nc.sync.dma_start(
                        xt0[t].rearrange("(a p) c -> p a c", p=128)[:, i4 * 4:(i4 + 1) * 4, :],
                        rstage[:])

        def xrow_gather(t):
            # emitted well after make_xt0 so this Pool gather never holds
            # Pool SEQ waiting for the xt0 table writes
            gat(xrow[t], xt0[t], W["myid"], 0, NBLK, 128)

        def emit_tables(t, l, psbufs=2):
            with tc.tile_pool(name="tab", bufs=2) as tp, \
                 tc.tile_pool(name="tabps", bufs=psbufs, space="PSUM") as tps:
                NH = NT // 4
                for hh in range(4):
                    kvq = tp.tile([128, NH, 256], BF, tag="kvq")
                    for i in range(NH):
                        g = hh * NH + i
                        ps = tps.tile([128, 256], dt.float32, tag="psT")
                        nc.tensor.matmul(ps[:], xc[t][:, bass.ts(g, 128)],
                                         W[f"Wtab{t}_l{l}"], start=True,
                                         stop=bias_zero)
                        if not bias_zero:
                            nc.tensor.matmul(ps[:], W["ones1"][:1, :],
                                             W[f"btab{t}_l{l}"][:1, :],
                                             start=False, stop=True)
                        cpy(nc.scalar if i % 2 else nc.vector,
                            kvq[:, i, :], ps[:])
                    nc.sync.dma_start(
                        kv[t].rearrange("(a p) c -> p a c", p=128)[:, hh * NH:(hh + 1) * NH, :],
                        kvq[:])

        def qmy_local(t, l):
            # q rows of MY nodes, straight from local node-major x rows:
            # no q table, no gather, no collective dependency
            with tc.tile_pool(name="qml", bufs=2) as qp, \
                 tc.tile_pool(name="qmlps", bufs=1, space="PSUM") as qps:
                for b in range(NBLK):
                    ptr = qps.tile([128, 128], BF, tag="qtr")
                    nc.tensor.transpose(ptr[:], xrow[t][:, b, :], W["ident"])
                    xnT = qp.tile([128, 128], BF, tag="xnT")
                    cpy(nc.scalar if b % 2 == 0 else nc.vector, xnT[:], ptr[:])
                    psQ = qps.tile([128, 128], dt.float32, tag="psQ")
                    nc.tensor.matmul(psQ[:], xnT[:], W[f"Wq{t}_l{l}"],
                                     start=True, stop=bias_zero)
                    if not bias_zero:
                        nc.tensor.matmul(psQ[:], W["ones1"][:1, :],
                                         W[f"bq{t}_l{l}"][:1, :],
                                         start=False, stop=True)
                    cpy(nc.scalar if b % 2 == 0 else nc.vector,
                        qmy[t][:, b, :], psQ[:])

        # ---------- edge phase pools (persistent across dirs/layers) ----------
        CH = 24
        estack = ExitStack()
        epools = {}

        def open_edge_pools():
            epools["eg"] = estack.enter_context(tc.tile_pool(name="eg", bufs=4))
            epools["epw"] = estack.enter_context(tc.tile_pool(name="epw", bufs=3))
            epools["ew"] = estack.enter_context(tc.tile_pool(name="ew", bufs=2))
            epools["eta"] = estack.enter_context(tc.tile_pool(name="eta", bufs=2))
            epools["eps"] = estack.enter_context(
                tc.tile_pool(name="eps", bufs=2, space="PSUM"))
            epools["epsg"] = estack.enter_context(
                tc.tile_pool(name="epsg", bufs=2, space="PSUM"))
            epools["epsu"] = estack.enter_context(
                tc.tile_pool(name="epsu", bufs=1, space="PSUM"))

        def edge_dir(dname, st, dtt, l, do_cc=True, mid=None, mid2=None,
                     early=None):
            eg = epools["eg"]; epw = epools["epw"]; ew = epools["ew"]
            eta = epools["eta"]; eps = epools["eps"]
            epsg = epools["epsg"]; epsu = epools["epsu"]
            Js = JL[dname]
            off8 = [0] * NBLK
            acc = 0
            for b in range(NBLK):
                off8[b] = acc
                acc += Js[b] * 8
            # blocks in descending-J order
            border = sorted(range(NBLK), key=lambda b: -Js[b])
            agf = eta.tile([128, NBLK, 128], dt.float32, tag="agf",
                           name=f"agf{dname}l{l}")
            rsa = eta.tile([128, NBLK, 8], dt.float32, tag="rsa",
                           name=f"rsa{dname}l{l}")
            agT = eta.tile([128, NBLK, 128], BF, tag="agT",
                           name=f"agT{dname}l{l}")

            def block_tail(b):
                # inline normalize+gelu(tanh)+update for one dst block
                gnt = ew.tile([128, 128], BF, tag="gn")
                tt(nc.vector, gnt[:].rearrange("p (d h) -> p d h", h=8),
                   agf[:, b, :].rearrange("p (d h) -> p d h", h=8),
                   rsa[:, b, :].rearrange("p (d2 h) -> p d2 h", d2=1), ALU.mult)
                t2 = ew.tile([128, 128], BF, tag="t2")
                nc.scalar.square(t2[:], gnt[:])
                u = ew.tile([128, 128], BF, tag="u")
                nc.vector.tensor_scalar(u[:], t2[:], 0.044715, 1.0,
                                        ALU.mult, ALU.add)
                v = ew.tile([128, 128], BF, tag="v")
                tt(nc.vector, v[:], gnt[:], u[:], ALU.mult)
                th = ew.tile([128, 128], BF, tag="th")
                nc.scalar.activation(th[:], v[:], AF.Tanh,
                                     scale=0.7978845608)
                hm = ew.tile([128, 128], BF, tag="hm")
                nc.vector.tensor_scalar(hm[:], th[:], 0.5, 0.5,
                                        ALU.mult, ALU.add)
                gb = ew.tile([128, 128], BF, tag="gb")
                tt(nc.vector, gb[:], gnt[:], hm[:], ALU.mult)
                trp = epsu.tile([128, 128], BF, tag="trp")
                nc.tensor.transpose(trp[:], gb[:], W["ident"])
                gT = ew.tile([128, 128], BF, tag="gT")
                nc.scalar.copy(gT[:], trp[:])
                psU = epsu.tile([128, 128], dt.float32, tag="psU")
                nc.tensor.matmul(psU[:], gT[:], W[f"Wup{dtt}_l{l}"],
                                 start=True, stop=False)
                if not bias_zero:
                    nc.tensor.matmul(psU[:], W["ones1"][:1, :],
                                     W[f"bup{dtt}_l{l}"][:1, :],
                                     start=False, stop=False)
                nc.tensor.matmul(psU[:], W[f"Ibl{dtt}_l{l}"],
                                 xrow[dtt][:, b, :], start=False, stop=True)
                nc.scalar.copy(xnew[dtt][:, b, :], psU[:])
                if do_cc and l == 0:
                    trpn = epsu.tile([128, 128], BF, tag="trp")
                    nc.tensor.transpose(trpn[:], xnew[dtt][:, b, :],
                                        W["ident"])
                    cpy(nc.scalar if b % 2 == 0 else nc.vector,
                        agT[:, b, :], trpn[:])

            def write_agin(b0, b1):
                # stage this half's cc input early; the collective itself is
                # emitted later so it never holds Pool SEQ waiting for input
                if not do_cc:
                    return
                if l == 0:
                    h = "a" if b0 else "b"
                    nc.sync.dma_start(aginTh[f"{dtt}{h}"][:, :],
                                      agT[:, b0:b1, :].rearrange(
                                          "p b c -> p (b c)"))
                else:
                    nc.sync.dma_start(
                        agin2b.rearrange("(b p) c -> p b c", p=128)[:, b0:b1, :],
                        xnew[dtt][:, b0:b1, :])

            def cc_only(h, b0, b1):
                if not do_cc:
                    return
                if l == 0:
                    nc.gpsimd.collective_compute(
                        "AllGather", mybir.AluOpType.bypass,
                        ins=[aginTh[f"{dtt}{h}"][:, :]],
                        outs=[agoutT[f"{dtt}{h}"][:]],
                        replica_groups=[list(range(NCORE))])
                elif h == "b":
                    # layer 1: one full collective (gates the final phase;
                    # splitting it only adds fixed overhead)
                    nc.gpsimd.collective_compute(
                        "AllGather", mybir.AluOpType.bypass,
                        ins=[agin2b[:]], outs=[agout2bf[:]],
                        replica_groups=[list(range(NCORE))])

            done = 0
            for b in border:
                J = Js[b]
                nch = (J + CH - 1) // CH
                psG = sacc = None
                for ci in range(nch):
                    j0 = ci * CH
                    jn = min(CH, J - j0)
                    first = ci == 0
                    last = ci == nch - 1
                    coff8 = off8[b] + j0 * 8
                    kvg = eg.tile([128, CH, 256], BF, tag="kvg")
                    gat(kvg, kv[st], W[f"ei{dname}"], coff8, jn, 256)
                    prod = epw.tile([128, CH, 128], BF, tag="pw", name="prod")
                    tt(nc.vector, prod[:, 0:jn, :], kvg[:, 0:jn, 0:128],
                       qmy[dtt][:, b:b + 1, :], ALU.mult)
                    # alpha[p, j, h] = sum_d prod[p, j, d*8+h]: PE
                    # identity-matmul accumulation over the 16 d-slabs
                    psA = eps.tile([128, CH * 8], dt.float32, tag="psA")
                    for dd in range(D):
                        nc.tensor.matmul(psA[:, 0:jn * 8], W["ident"],
                                         prod[:, 0:jn, bass.ts(dd, 8)],
                                         start=(dd == 0), stop=(dd == D - 1))
                    eB = ew.tile([128, CH, 8], BF, tag="eB")
                    nc.scalar.activation(
                        eB[:, 0:jn, :],
                        psA[:, 0:jn * 8].rearrange("p (j h) -> p j h", h=8),
                        AF.Exp)
                    # wv[p, j, d*8+h] = v * e  (2x: d-major v, h innermost)
                    wv = epw.tile([128, CH, 128], BF, tag="pw", name="wv")
                    tt(nc.vector,
                       wv[:, 0:jn, :].rearrange("p j (d h) -> p j d h", h=8),
                       kvg[:, 0:jn, 128:256].rearrange("p j (d h) -> p j d h", h=8),
                       eB[:, 0:jn, :].rearrange("p j (d2 h) -> p j d2 h", d2=1),
                       ALU.mult)
                    # s tree over j (in-place on eB), bf16 accum
                    with nc.allow_low_precision(reason="softmax denom bf16 tree"):
                        Jc = jn
                        while Jc > 1:
                            h1 = (Jc + 1) // 2
                            tt(nc.vector, eB[:, 0:Jc - h1, :], eB[:, 0:Jc - h1, :],
                               eB[:, h1:Jc, :], ALU.add)
                            Jc = h1
                    if first:
                        sacc = ew.tile([128, 8], dt.float32, tag="sacc")
                        nc.vector.tensor_copy(sacc[:], eB[:, 0, :])
                    else:
                        tt(nc.vector, sacc[:], sacc[:], eB[:, 0, :], ALU.add)
                    # agg[p, dh] += sum_j wv: 4-tile-packed identity matmuls
                    if first:
                        psG = epsg.tile([128, 4, 128], dt.float32, tag="psG")
                    nst = (jn + 3) // 4
                    for g in range(nst):
                        gw = min(4, jn - g * 4)
                        nc.tensor.matmul(psG[:, 0:gw, :], W["ident"],
                                         wv[:, g * 4:g * 4 + gw, :],
                                         start=(first and g == 0),
                                         stop=(last and g == nst - 1))
                    if not last:
                        continue
                    sden = ew.tile([128, 8], dt.float32, tag="sden")
                    tt(nc.vector, sden[:, :], sacc[:],
                       W[f"pc{dname}"][:, b:b + 1], ALU.subtract)
                    nc.vector.reciprocal(rsa[:, b, :], sden[:])
                    nc.vector.tensor_reduce(
                        agf[:, b, :], psG[:].rearrange("p r c -> p c r"),
                        mybir.AxisListType.X, ALU.add)
                block_tail(b)
                done += 1
                if done == 2 and early is not None:
                    early()
                if done == NBLK // 2:
                    write_agin(NBLK // 2, NBLK)
                    if mid is not None:
                        mid()
                if done == NBLK // 2 + 2 and l == 0:
                    # input staged 2 blocks ago -> no Pool SEQ hold here
                    cc_only("a", NBLK // 2, NBLK)
                if done == NBLK - 1 and mid2 is not None:
                    mid2()
            write_agin(0, NBLK // 2)
            if dtt == 1:
                # my updated type-1 rows into the local final table
                nc.sync.dma_start(
                    xtloc.rearrange("(b p) c -> p b c", p=128)[:, :, l * 128:(l + 1) * 128],
                    xnew[1][:])
            return (lambda: cc_only("b", 0, NBLK // 2)) if do_cc else None

        def post_xc(t):
            # xc[t] column order IS the AllGather row order, so the rebuild
            # is 16 plain contiguous reads straight into the xc tile
            for h, b0 in (("a", NBLK // 2), ("b", 0)):
                for r in range(NCORE):
                    nc.sync.dma_start(
                        xc[t][:, r * NLOC + b0 * 128:
                              r * NLOC + b0 * 128 + HB],
                        agoutT[f"{t}{h}"][r])

        def emed_l0_from_xc():
            # final ed table's layer-0 columns = node-major transposes of
            # xc2 (runs in layer-1 slack, off every critical path)
            with tc.tile_pool(name="eml0", bufs=3) as pp, \
                 tc.tile_pool(name="eml0ps", bufs=2, space="PSUM") as ppp:
                emv = emed2c[0:NCORE * NLOC, 0:128].rearrange(
                    "(g p) c -> p g c", p=128)
                for i4 in range(NT // 4):
                    ptr = ppp.tile([128, 4, 128], BF, tag="ptr")
                    for k in range(4):
                        nc.tensor.transpose(
                            ptr[:, k, :],
                            xc[2][:, bass.ts(i4 * 4 + k, 128)], W["ident"])
                    rstage = pp.tile([128, 4, 128], BF, tag="rst")
                    cpy(nc.scalar if i4 % 2 == 0 else nc.vector,
                        rstage[:], ptr[:])
                    nc.sync.dma_start(emv[:, i4 * 4:(i4 + 1) * 4, :],
                                      rstage[:])

        def copy_l1_half():
            # stage agout2bf into the final ed table's layer-1 columns
            with tc.tile_pool(name="cl1", bufs=3) as pp:
                emv = emed2c[0:NCORE * NLOC, 128:256].rearrange(
                    "(g p) c -> p g c", p=128)
                srcv = agout2bf[:].rearrange("r (b p) c -> p (r b) c",
                                             p=128)
                for q4 in range(NT // 4):
                    rd = pp.tile([128, 4, 128], BF, tag="rd")
                    nc.sync.dma_start(rd[:], srcv[:, q4 * 4:(q4 + 1) * 4, :])
                    nc.sync.dma_start(emv[:, q4 * 4:(q4 + 1) * 4, :], rd[:])

        # ---------- layers: phase stamps act as scheduler barriers; keep
        # one only where collective-gated work could otherwise poison an
        # independent engine stream ----------
        tc.tile_set_cur_wait(1)
        make_xc(2)
        emit_tables(2, 0, psbufs=4)
        make_xc(1)
        make_xt0(1)
        xrow_gather(1)
        qmy_local(1, 0)
        p0stack.close()
        open_edge_pools()
        tc.tile_set_cur_wait(2)
        emit_tables(1, 0)
        cc1b = edge_dir("21", 2, 1, 0,
                        mid=lambda: make_xt0(2),
                        mid2=lambda: (xrow_gather(2), qmy_local(2, 0)))
        cc2b = edge_dir("12", 1, 2, 0, early=cc1b)
        xrow, xnew = xnew, xrow
        # the cc-gated packs stay in the PREVIOUS phase: their gates
        # complete mid-direction, so they fill the direction's drain
        cc2b()
        post_xc(1)
        emit_tables(1, 1)
        qmy_local(2, 1)
        qmy_local(1, 1)
        tc.tile_set_cur_wait(3)
        cc2bb = edge_dir("12", 1, 2, 1)
        cc2bb()
        post_xc(2)
        emit_tables(2, 1)
        emed_l0_from_xc()
        tc.tile_set_cur_wait(4)
        edge_dir("21", 2, 1, 1, do_cc=False)
        copy_l1_half()
        tc.tile_set_cur_wait(5)
        estack.close()

        # ---------- final gather-dot (m-grouped, local dense Em side) ----------
        with tc.tile_pool(name="fin", bufs=4) as fp, \
             tc.tile_pool(name="fpb", bufs=3) as fpb, \
             tc.tile_pool(name="fem", bufs=1) as fem, \
             tc.tile_pool(name="finps", bufs=4, space="PSUM") as fps, \
             tc.tile_pool(name="ybuf", bufs=1) as yp:
            ysb = yp.tile([128, YC], dt.float32, tag="ysb")
            emT = fem.tile([128, NFT, 256], BF, tag="emT")
            gat(emT, xtloc, W["emlid"], 0, NFT, 256)
            col = 0
            for g in range(NFT):
                base8 = sum(JF[:g]) * 8
                for j0 in range(0, JF[g], 8):
                    gn_t = min(8, JF[g] - j0)
                    ed = fp.tile([128, 8, 256], BF, tag="ed")
                    gat(ed, emed2c, W["fei"], base8 + j0 * 8, gn_t, 256)
                    pb = fpb.tile([128, 8, 256], BF, tag="pb")
                    tt(nc.vector, pb[:, 0:gn_t, :], ed[:, 0:gn_t, :],
                       emT[:, g:g + 1, :], ALU.mult)
                    # slab-sum on PE: psY[p, t, i] = sum_s pb[p, t, s*16+i]
                    psY = fps.tile([128, 8, 16], dt.float32, tag="psY")
                    for s in range(16):
                        nc.tensor.matmul(psY[:, 0:gn_t, :], W["ident"],
                                         pb[:, 0:gn_t, bass.ts(s, 16)],
                                         start=(s == 0), stop=(s == 15))
                    nc.vector.tensor_reduce(
                        ysb[:, col:col + gn_t], psY[:, 0:gn_t, :],
                        mybir.AxisListType.X, ALU.add)
                    col += gn_t
            nc.sync.dma_start(y_out[:, :], ysb[:])
    nc.compile()
    return nc


_CACHE = {}
_last_key = None


def kernel(**inputs):
    global _last_key
    from concourse.bass_utils import run_bass_kernel_spmd
    P, e12, e21, fin, J12, J21, JF = _host_prep(inputs)
    bz = all(not np.any(np.asarray(inputs[k]))
             for k in inputs if k.startswith("b"))
    key = (J12, J21, JF, bz)
    _last_key = key
    if key not in _CACHE:
        _CACHE[key] = _build(J12, J21, JF, bias_zero=bz)
    nc = _CACHE[key]
    in_maps = []
    for c in range(NCORE):
        m = dict(P)
        m["ei12"] = e12[c]["idx"]; m["pc12"] = e12[c]["padc"]
        m["ei21"] = e21[c]["idx"]; m["pc21"] = e21[c]["padc"]
        m["myid"] = fin[c]["myid"]
        m["emlid"] = fin[c]["emlid"]; m["fei"] = fin[c]["fei"]
        in_maps.append(m)
    res = run_bass_kernel_spmd(nc, in_maps, list(range(NCORE)))
    y = np.zeros((EF,), np.float32)
    for c in range(NCORE):
        yc = np.asarray(res.results[c]["y"])      # [128, YC]
        ylin = yc.T.ravel()                        # slot (col, p) order
        pos = fin[c]["pos"]
        mreal = pos >= 0
        y[pos[mreal]] = ylin[mreal]
    return y.reshape(EF, 1)


# revision 34
# speedup vs baseline: 1.0268x; 1.0082x over previous
"""Trainium2 Bass kernel for 2-layer HGT message passing + sparse gather-dot,
sharded over 8 NeuronCores.

Layout strategy (v3):
 - Nodes of each type are RELABELED host-side by in-degree rank:
   new_id = band*128 + slot, band = rank//128 (80 bands, degree-sorted),
   core(band) = band % 8.  All indices (edges, final queries) are remapped
   through the permutation, so the device never sees it.
 - Edge phase uses a dst-per-partition layout: for a 128-dst block, slot
   (p, j) holds the j-th in-edge of dst p.  J_b = max in-block degree is a
   compile-time constant per block.  Blocks run in descending-J order so
   the heavy block's gathers prefetch first and the pipeline tail is the
   smallest block.
 - gelu+update tails are deferred to the end of each direction so the
   Activation engine swaps tables (Exp<->Gelu) only twice per direction.
 - The per-type AllGather writes DIRECTLY into the emed staging table via
   a strided output AP (no stage readback / emed rewrite).
 - Final gather-dot: queries are grouped by their m-node's OWNER core and
   packed into query-count-sorted 128-node tiles (slot (p, j) = j-th query
   of m-node p).  The dense Em side comes from a core-LOCAL table (both
   layers' outputs, written without any collective), so the layer-1 type-1
   AllGather is not needed at all (3 collectives, not 4).  Only Ed rows
   are gathered per query (from emed2, gated on the EARLY layer-1
   collective) -> half the final gather traffic of the pair-gather scheme
   and no collective on the final critical path.
 - All tables and gathered data are bf16 (512B gather rows).  PSUM stays
   f32.
"""
import numpy as np

N = 10000
NP = 10240          # padded node count (80 tiles of 128)
NT = NP // 128      # 80 tiles
NCORE = 8
NBLK = NT // NCORE  # 10 blocks (dst tiles) per core
NLOC = NBLK * 128   # 1280 nodes owned per core
F = 128; HID = 128; H = 8; D = 16; L = 2
EF = 500000
NFT = NBLK          # 10 final m-tiles per core
ZROW = NP           # zero row in kv/emed tables used by padding slots


def _wrap_idx(idx):
    """int index list (len%16==0) -> [128, len//16] int16 in gather format."""
    a = np.asarray(idx, np.int16).reshape(-1, 16).T
    return np.ascontiguousarray(np.tile(a, (8, 1)))


def _blockdiag(a):
    out = np.zeros((HID, HID), np.float32)
    for h in range(H):
        out[h * D:(h + 1) * D, h * D:(h + 1) * D] = a[h]
    return out


# column permutation (h,d) -> d-major (d*8+h)
_PDH = np.zeros(HID, np.int64)
for _h in range(H):
    for _d in range(D):
        _PDH[_d * H + _h] = _h * D + _d   # new col i=d*8+h takes old col h*16+d


def _perm_from_degree(deg):
    """deg[NP] -> perm (old->new), degree-ascending bands dealt round-robin."""
    order = np.argsort(deg, kind="stable")       # order[r] = old id
    perm = np.empty(NP, np.int64)
    r = np.arange(NP)
    perm[order] = r                               # new_id = rank
    return perm


def _rmap(x):
    """permuted id -> table row in AllGather output order (r, b, p)."""
    band = x // 128
    return ((band % NCORE) * NBLK + band // NCORE) * 128 + x % 128


_RINV = None


def _rinv():
    global _RINV
    if _RINV is None:
        inv = np.empty(NP, np.int64)
        inv[_rmap(np.arange(NP))] = np.arange(NP)
        _RINV = inv
    return _RINV


def _prep_edges(ei, perm_s, perm_d):
    """-> per-core dict(idx [128, SJ*8] i16, padc [NBLK,128] f32), J list."""
    s = _rmap(perm_s[np.asarray(ei[0])])
    d = perm_d[np.asarray(ei[1])]
    band = d // 128
    core = band % NCORE
    blk = band // NCORE
    p = d % 128
    # j-th edge of each dst: stable sort by d, position within group
    order = np.argsort(d, kind="stable")
    ds = d[order]
    cnt = np.bincount(d, minlength=NP)
    starts = np.zeros(NP + 1, np.int64)
    np.cumsum(cnt, out=starts[1:])
    j_of = np.arange(len(ds)) - starts[ds]
    # J per (core, blk): max degree in band
    J = np.zeros((NCORE, NBLK), np.int64)
    for b in range(NT):
        mx = cnt[b * 128:(b + 1) * 128].max()
        J[b % NCORE, b // NCORE] = max(J[b % NCORE, b // NCORE], mx)
    Jb = [max(1, int(J[:, b].max())) for b in range(NBLK)]  # same for all cores
    out = []
    ss = s[order]
    cs = core[order]; bs = blk[order]; ps = p[order]
    for c in range(NCORE):
        idxs = []
        padc = np.zeros((NBLK, 128), np.float32)
        m_c = cs == c
        for b in range(NBLK):
            Jcb = Jb[b]
            A = np.full((Jcb, 128), ZROW, np.int64)
            m = m_c & (bs == b)
            A[j_of[m], ps[m]] = ss[m]
            band_cnt = cnt[(b * NCORE + c) * 128:(b * NCORE + c + 1) * 128]
            # 1e-3 denominator bias keeps zero-degree rows finite (0*1000=0);
            # relative effect on real weights ~1e-3/32, far under tolerance
            padc[b, :] = (Jcb - band_cnt).astype(np.float32) - 1e-3
            idxs.append(_wrap_idx(A.reshape(-1)))
        out.append({"idx": np.ascontiguousarray(np.hstack(idxs)),
                    "padc": padc})
    return out, Jb


def _prep_final(eidx, perm1, perm2):
    """Queries grouped by m-node owner core; count-sorted local tiles.

    Returns per-core dicts (emlid: local m ids per tile for the dense Em
    gather, fei: ed gather idx per slot, pos: slot -> original query id or
    -1) and JF (per-tile max query count, shared across cores).
    """
    mi = perm1[np.asarray(eidx[0])]
    di = perm2[np.asarray(eidx[1])]
    band = mi // 128
    cq = band % NCORE
    lid = (band // NCORE) * 128 + mi % 128       # local row in xtloc
    # ed side reads the emed2c table whose rows are in AllGather output
    # order (r, b, p)
    di = _rmap(di)
    key = cq * NLOC + lid
    cntq = np.bincount(key, minlength=NCORE * NLOC).reshape(NCORE, NLOC)
    rk = np.empty((NCORE, NLOC), np.int64)
    sorted_cnt = np.empty((NCORE, NLOC), np.int64)
    emlid = np.empty((NCORE, NLOC), np.int64)
    for c in range(NCORE):
        o = np.argsort(-cntq[c], kind="stable")
        rk[c, o] = np.arange(NLOC)
        sorted_cnt[c] = cntq[c][o]
        emlid[c] = o                              # rank -> local id
    JF = [max(1, int(sorted_cnt[:, g * 128].max())) for g in range(NFT)]
    # position of each query within its (core, m-node) group
    order = np.argsort(key, kind="stable")
    ks = key[order]
    starts = np.zeros(NCORE * NLOC + 1, np.int64)
    np.cumsum(cntq.reshape(-1), out=starts[1:])
    j_of = np.arange(EF) - starts[ks]
    ds = di[order]; oq = order
    rq = rk[cq[order], lid[order]]
    gq = rq // 128
    pq = rq % 128
    cs = cq[order]
    percore = []
    for c in range(NCORE):
        m_c = cs == c
        A_all = []
        P_all = []
        for g in range(NFT):
            A = np.full((JF[g], 128), NCORE * NLOC, np.int64)
            POS = np.full((JF[g], 128), -1, np.int64)
            m = m_c & (gq == g)
            A[j_of[m], pq[m]] = ds[m]
            POS[j_of[m], pq[m]] = oq[m]
            A_all.append(A)
            P_all.append(POS)
        fei = np.vstack(A_all)            # [sum(JF), 128]
        pos = np.vstack(P_all).reshape(-1)
        percore.append({"emlid": _wrap_idx(emlid[c]),
                        "fei": _wrap_idx(fei.reshape(-1)),
                        "pos": pos})
    return percore, tuple(JF)


def _host_prep(inp):
    f32 = lambda x: np.asarray(x, np.float32)
    ei12 = np.asarray(inp["ei_12"]); ei21 = np.asarray(inp["ei_21"])
    deg1 = np.bincount(np.asarray(ei21[1]), minlength=NP)[:NP]
    deg2 = np.bincount(np.asarray(ei12[1]), minlength=NP)[:NP]
    perm = {1: _perm_from_degree(deg1), 2: _perm_from_degree(deg2)}
    inv = {t: np.argsort(perm[t]) for t in (1, 2)}

    P = {}
    for t, xn, wn, bn in ((1, "x_n1", "W_in1", "b_in1"), (2, "x_n2", "W_in2", "b_in2")):
        x = np.zeros((NP, F), np.float32)
        x[:N] = f32(inp[xn])
        import ml_dtypes
        P[f"xT{t}"] = np.ascontiguousarray(
            x[inv[t]][_rinv()].T.astype(ml_dtypes.bfloat16))
        P[f"Win{t}"] = f32(inp[wn])
        P[f"binc{t}"] = np.ascontiguousarray(f32(inp[bn]).reshape(HID, 1))
        P[f"binr{t}"] = f32(inp[bn]).reshape(1, HID)
    for t in (1, 2):
        rel = "12" if t == 1 else "21"
        sfx = f"n{t}"
        for l in range(L):
            bd_a = _blockdiag(f32(inp[f"a_rel_{rel}"][l]))
            bd_m = _blockdiag(f32(inp[f"m_rel_{rel}"][l]))
            scale = np.repeat(f32(inp[f"p_rel_{rel}"][l]), D) / np.sqrt(D)
            wk = (f32(inp[f"Wk_{sfx}"][l]) @ bd_a * scale[None, :])[:, _PDH]
            bk = (f32(inp[f"bk_{sfx}"][l]) @ bd_a * scale)[_PDH]
            wv = (f32(inp[f"Wv_{sfx}"][l]) @ bd_m)[:, _PDH]
            bv = (f32(inp[f"bv_{sfx}"][l]) @ bd_m)[_PDH]
            wq = f32(inp[f"Wq_{sfx}"][l])[:, _PDH]
            bq = f32(inp[f"bq_{sfx}"][l])[_PDH]
            P[f"Wtab{t}_l{l}"] = np.ascontiguousarray(
                np.concatenate([wk, wv], axis=1))                # [128, 256]
            P[f"btab{t}_l{l}"] = np.concatenate([bk, bv]).reshape(1, 2 * HID)
            P[f"Wq{t}_l{l}"] = np.ascontiguousarray(wq)
            P[f"bq{t}_l{l}"] = bq.reshape(1, HID)
            b = 1.0 / (1.0 + np.exp(-float(inp[f"skip_{sfx}"][l])))
            P[f"Wup{t}_l{l}"] = np.ascontiguousarray(b * f32(inp[f"Wa_{sfx}"][l])[_PDH, :])
            P[f"bup{t}_l{l}"] = (b * f32(inp[f"ba_{sfx}"][l])).reshape(1, HID)
            P[f"Ibl{t}_l{l}"] = ((1.0 - b) * np.eye(HID)).astype(np.float32)
    P["ident"] = np.eye(128, dtype=np.float32)
    P["ones1"] = np.ones((1, 128), np.float32)
    packs = [P.pop("Win1"), P.pop("Win2"), P.pop("ident")]
    for t in (1, 2):
        for l in range(L):
            packs.append(P.pop(f"Wtab{t}_l{l}"))
            packs.append(P.pop(f"Wq{t}_l{l}"))
    for t in (1, 2):
        for l in range(L):
            packs.append(P.pop(f"Wup{t}_l{l}"))
            packs.append(P.pop(f"Ibl{t}_l{l}"))
    P["wpack"] = np.ascontiguousarray(np.concatenate(packs, axis=1))
    P["bincp"] = np.ascontiguousarray(
        np.concatenate([P.pop("binc1"), P.pop("binc2")], axis=1))

    e12, J12 = _prep_edges(ei12, perm[1], perm[2])
    e21, J21 = _prep_edges(ei21, perm[2], perm[1])
    fin, JF = _prep_final(np.asarray(inp["edge_index"]), perm[1], perm[2])

    # per-core my-node rows: contiguous range in R (r,b,p) row order
    for c in range(NCORE):
        fin[c]["myid"] = _wrap_idx(np.arange(c * NLOC, (c + 1) * NLOC))
    return P, e12, e21, fin, tuple(J12), tuple(J21), JF


def _build(J12, J21, JF, bias_zero=False, gcap=8, scratch=16384):
    import concourse.bass as bass
    import concourse.mybir as mybir
    from concourse import bacc, tile, library_config
    from concourse.bass import broadcast_tensor_aps

    dt = mybir.dt
    AF = mybir.ActivationFunctionType
    ALU = mybir.AluOpType
    BF = dt.bfloat16
    nc = bacc.Bacc("TRN2", dynamic_dma_scratch_size=scratch)

    SJ8 = {d: sum(J) * 8 for d, J in (("12", J12), ("21", J21))}
    JL = {"12": J12, "21": J21}
    SJF = sum(JF)
    YC = SJF

    def inP(name, shape, dty=dt.float32):
        return nc.declare_dram_parameter(name, list(shape), dty, isOutput=False)

    WCOLS = 128 * 3 + 384 * L * 2 + 256 * L * 2  # same total, [k|v]+q split
    pr = {}
    for t in (1, 2):
        pr[f"xT{t}"] = inP(f"xT{t}", [128, NP], BF)
        pr[f"binr{t}"] = inP(f"binr{t}", [1, 128])
        for l in range(L):
            for nm, sh in (("btab", [1, 256]), ("bq", [1, 128]),
                           ("bup", [1, 128])):
                pr[f"{nm}{t}_l{l}"] = inP(f"{nm}{t}_l{l}", sh)
    pr["wpack"] = inP("wpack", [128, WCOLS])
    pr["bincp"] = inP("bincp", [128, 2])
    pr["ones1"] = inP("ones1", [1, 128])
    for dname in ("12", "21"):
        pr[f"ei{dname}"] = inP(f"ei{dname}", [128, SJ8[dname]], dt.int16)
        pr[f"pc{dname}"] = inP(f"pc{dname}", [NBLK, 128])
    pr["myid"] = inP("myid", [128, NBLK * 8], dt.int16)
    pr["emlid"] = inP("emlid", [128, NFT * 8], dt.int16)
    pr["fei"] = inP("fei", [128, SJF * 8], dt.int16)
    y_out = nc.declare_dram_parameter("y", [128, YC], dt.float32, isOutput=True)

    kv = {t: nc.dram_tensor(f"kv{t}", [NP + 128, 256], BF) for t in (1, 2)}
    xt0 = {t: nc.dram_tensor(f"xt0{t}", [NP, 128], BF) for t in (1, 2)}
    HB = NLOC // 2
    agoutT = {f"{k}{h}": nc.dram_tensor(f"agoutT{k}{h}", [NCORE, 128, HB], BF,
                                        addr_space="Shared")
              for k in ("1", "2") for h in ("a", "b")}
    agout2bf = nc.dram_tensor("agout2bf", [NCORE, NLOC, 128], BF,
                              addr_space="Shared")
    emed2c = nc.dram_tensor("emed2c", [NCORE * NLOC + 128, 256], BF)
    aginTh = {f"{t}{h}": nc.dram_tensor(f"aginT{t}{h}", [128, HB], BF)
              for t in (1, 2) for h in ("a", "b")}
    agin2b = nc.dram_tensor("agin2b", [NLOC, 128], BF)
    xtloc = nc.dram_tensor("xtloc", [NLOC, 256], BF)   # my m rows, both layers

    from contextlib import ExitStack
    with tile.TileContext(nc) as tc, ExitStack() as stack:
        nc.gpsimd.load_library(library_config.mlp)
        cp = stack.enter_context(tc.tile_pool(name="const", bufs=1))
        W = {}
        # f32 weights -> bf16 SBUF copies
        with tc.tile_pool(name="wld", bufs=1) as wp:
            wf = wp.tile([128, WCOLS], dt.float32, tag="wf")
            nc.sync.dma_start(wf[:], pr["wpack"][:])
            wb = cp.tile([128, WCOLS], BF, tag="wb")
            nc.vector.tensor_copy(wb[:], wf[:])
            off = 0
            names = ["Win1", "Win2", "ident"]
            widths = [128, 128, 128]
            for t in (1, 2):
                for l in range(L):
                    names.append(f"Wtab{t}_l{l}"); widths.append(256)
                    names.append(f"Wq{t}_l{l}"); widths.append(128)
            for t in (1, 2):
                for l in range(L):
                    names.append(f"Wup{t}_l{l}"); widths.append(128)
                    names.append(f"Ibl{t}_l{l}"); widths.append(128)
            for nm, wd in zip(names, widths):
                W[nm] = wb[:, off:off + wd]
                off += wd
            bc = cp.tile([128, 2], dt.float32, tag="bincp")
            nc.sync.dma_start(bc[:], pr["bincp"][:])
            W["binc1"] = bc[:, 0:1]
            W["binc2"] = bc[:, 1:2]
            if not bias_zero:
                for k in ("ones1", "binr1", "binr2",
                          *(f"btab{t}_l{l}" for t in (1, 2) for l in range(L)),
                          *(f"bq{t}_l{l}" for t in (1, 2) for l in range(L)),
                          *(f"bup{t}_l{l}" for t in (1, 2) for l in range(L))):
                    p = pr[k]
                    tf2 = wp.tile(list(p.shape), dt.float32,
                                  tag="wf1" + str(list(p.shape)), bufs=2)
                    nc.sync.dma_start(tf2[:], p[:])
                    t_ = cp.tile(list(p.shape), BF, tag=k)
                    nc.vector.tensor_copy(t_[:], tf2[:])
                    W[k] = t_
        for dname in ("12", "21"):
            t_ = cp.tile([128, SJ8[dname]], dt.int16, tag=f"ei{dname}")
            nc.sync.dma_start(t_[:], pr[f"ei{dname}"][:])
            W[f"ei{dname}"] = t_
            t_ = cp.tile([128, NBLK], dt.float32, tag=f"pc{dname}")
            nc.sync.dma_start(t_[:], pr[f"pc{dname}"].rearrange("b p -> p b"))
            W[f"pc{dname}"] = t_
        for k, wd in (("myid", NBLK * 8), ("emlid", NFT * 8), ("fei", SJF * 8)):
            t_ = cp.tile([128, wd], dt.int16, tag=k)
            nc.sync.dma_start(t_[:], pr[k][:])
            W[k] = t_

        def tt(eng, out, a, b, op):
            a2, b2 = broadcast_tensor_aps(a, b)
            eng.tensor_tensor(out, a2, b2, op)

        def cpy(eng, dst, src):
            if eng is nc.scalar:
                eng.copy(dst, src)
            else:
                eng.tensor_copy(dst, src)

        def gat(out_t, table, idx_sb, base8, ntiles, elem):
            # dma_gather cap: gcap tiles (gcap*128 descriptors) per call
            for g0 in range(0, ntiles, gcap):
                gn = min(gcap, ntiles - g0)
                nc.gpsimd.dma_gather(
                    out_t[:, g0:g0 + gn, :], table[:, :],
                    idx_sb[:, base8 + g0 * 8:base8 + (g0 + gn) * 8],
                    gn * 128, gn * 128, elem)

        # persistent feature tiles
        xc = {t: cp.tile([128, NP], BF, tag=f"xc{t}", name=f"xc{t}") for t in (1, 2)}
        xrow = {t: cp.tile([128, NBLK, 128], BF, tag=f"xrow{t}", name=f"xrow{t}") for t in (1, 2)}
        xnew = {t: cp.tile([128, NBLK, 128], BF, tag=f"xnew{t}", name=f"xnew{t}") for t in (1, 2)}
        qmy = {t: cp.tile([128, NBLK, 128], BF, tag=f"qmy{t}", name=f"qmy{t}") for t in (1, 2)}

        # ---------- phase 0 part A: load x, project to xc ----------
        p0stack = ExitStack()
        p0 = p0stack.enter_context(tc.tile_pool(name="p0", bufs=2))
        p0b = p0stack.enter_context(tc.tile_pool(name="p0b", bufs=1))
        p0s = p0stack.enter_context(tc.tile_pool(name="p0s", bufs=3))
        p0p = p0stack.enter_context(tc.tile_pool(name="p0ps", bufs=4, space="PSUM"))
        xb = {}
        for t in (2, 1):
            xb[t] = p0b.tile([128, NP], BF, tag=f"xb{t}", name=f"xb{t}")
            for hf in range(4):
                nc.sync.dma_start(xb[t][:, bass.ts(hf, NP // 4)],
                                  pr[f"xT{t}"][:, bass.ts(hf, NP // 4)])
        # zero pad rows of kv + emed (kv zero row is read by the first dir)
        z = p0s.tile([128, 256], BF, tag="z")
        nc.vector.memset(z[:], 0.0)
        for t in (1, 2):
            nc.sync.dma_start(kv[t][NP:NP + 128, :], z[:])
        nc.sync.dma_start(emed2c[NCORE * NLOC:NCORE * NLOC + 128, :], z[:])

        def make_xc(t):
            # transposed projection -> xc (xT); relu split DVE/Act
            for j in range(NP // 512):
                ps = p0p.tile([128, 512], dt.float32, tag="psP")
                nc.tensor.matmul(ps[:], W[f"Win{t}"], xb[t][:, bass.ts(j, 512)],
                                 start=True, stop=True)
                if j % 2 == 0:
                    nc.vector.tensor_scalar(
                        xc[t][:, bass.ts(j, 512)], ps[:],
                        W[f"binc{t}"], 0.0, ALU.add, ALU.max)
                else:
                    nc.scalar.activation(xc[t][:, bass.ts(j, 512)], ps[:],
                                         AF.Relu, bias=W[f"binc{t}"])

        def make_xt0(t):
            # xt0 = node-major x0, built by transposing xc tiles (xc is
            # already relu'd + biased)
            with tc.tile_pool(name="p0r", bufs=3) as rp, \
                 tc.tile_pool(name="p0rps", bufs=2, space="PSUM") as rps:
                for i4 in range(NT // 4):
                    ptr = rps.tile([128, 4, 128], BF, tag="ptr")
                    for k in range(4):
                        nc.tensor.transpose(
                            ptr[:, k, :], xc[t][:, bass.ts(i4 * 4 + k, 128)],
                            W["ident"])
                    rstage = rp.tile([128, 4, 128], BF, tag="rst")
                    cpy(nc.scalar if i4 % 2 == 0 else nc.vector,
                        rstage[:], ptr[:])
                    nc.sync.dma_start(
                        xt0[t].rearrange("(a p) c -> p a c", p=128)[:, i4 * 4:(i4 + 1) * 4, :],
                        rstage[:])

        def xrow_gather(t):
            # emitted well after make_xt0 so this Pool gather never holds
            # Pool SEQ waiting for the xt0 table writes
            gat(xrow[t], xt0[t], W["myid"], 0, NBLK, 128)

        def emit_tables(t, l, psbufs=2):
            with tc.tile_pool(name="tab", bufs=2) as tp, \
                 tc.tile_pool(name="tabps", bufs=psbufs, space="PSUM") as tps:
                NH = NT // 4
                for hh in range(4):
                    kvq = tp.tile([128, NH, 256], BF, tag="kvq")
                    for i in range(NH):
                        g = hh * NH + i
                        ps = tps.tile([128, 256], dt.float32, tag="psT")
                        nc.tensor.matmul(ps[:], xc[t][:, bass.ts(g, 128)],
                                         W[f"Wtab{t}_l{l}"], start=True,
                                         stop=bias_zero)
                        if not bias_zero:
                            nc.tensor.matmul(ps[:], W["ones1"][:1, :],
                                             W[f"btab{t}_l{l}"][:1, :],
                                             start=False, stop=True)
                        cpy(nc.scalar if i % 2 else nc.vector,
                            kvq[:, i, :], ps[:])
                    nc.sync.dma_start(
                        kv[t].rearrange("(a p) c -> p a c", p=128)[:, hh * NH:(hh + 1) * NH, :],
                        kvq[:])

        def qmy_local(t, l):
            # q rows of MY nodes, straight from local node-major x rows:
            # no q table, no gather, no collective dependency
            with tc.tile_pool(name="qml", bufs=2) as qp, \
                 tc.tile_pool(name="qmlps", bufs=1, space="PSUM") as qps:
                for b in range(NBLK):
                    ptr = qps.tile([128, 128], BF, tag="qtr")
                    nc.tensor.transpose(ptr[:], xrow[t][:, b, :], W["ident"])
                    xnT = qp.tile([128, 128], BF, tag="xnT")
                    cpy(nc.scalar if b % 2 == 0 else nc.vector, xnT[:], ptr[:])
                    psQ = qps.tile([128, 128], dt.float32, tag="psQ")
                    nc.tensor.matmul(psQ[:], xnT[:], W[f"Wq{t}_l{l}"],
                                     start=True, stop=bias_zero)
                    if not bias_zero:
                        nc.tensor.matmul(psQ[:], W["ones1"][:1, :],
                                         W[f"bq{t}_l{l}"][:1, :],
                                         start=False, stop=True)
                    cpy(nc.scalar if b % 2 == 0 else nc.vector,
                        qmy[t][:, b, :], psQ[:])

        # ---------- edge phase pools (persistent across dirs/layers) ----------
        CH = 24
        estack = ExitStack()
        epools = {}

        def open_edge_pools():
            epools["eg"] = estack.enter_context(tc.tile_pool(name="eg", bufs=5))
            epools["epw"] = estack.enter_context(tc.tile_pool(name="epw", bufs=3))
            epools["ew"] = estack.enter_context(tc.tile_pool(name="ew", bufs=2))
            epools["eta"] = estack.enter_context(tc.tile_pool(name="eta", bufs=2))
            epools["eps"] = estack.enter_context(
                tc.tile_pool(name="eps", bufs=2, space="PSUM"))
            epools["epsg"] = estack.enter_context(
                tc.tile_pool(name="epsg", bufs=2, space="PSUM"))
            epools["epsu"] = estack.enter_context(
                tc.tile_pool(name="epsu", bufs=1, space="PSUM"))

        def edge_dir(dname, st, dtt, l, do_cc=True, mid=None, mid2=None,
                     early=None):
            eg = epools["eg"]; epw = epools["epw"]; ew = epools["ew"]
            eta = epools["eta"]; eps = epools["eps"]
            epsg = epools["epsg"]; epsu = epools["epsu"]
            Js = JL[dname]
            off8 = [0] * NBLK
            acc = 0
            for b in range(NBLK):
                off8[b] = acc
                acc += Js[b] * 8
            # blocks in descending-J order
            border = sorted(range(NBLK), key=lambda b: -Js[b])
            agf = eta.tile([128, NBLK, 128], dt.float32, tag="agf",
                           name=f"agf{dname}l{l}")
            rsa = eta.tile([128, NBLK, 8], dt.float32, tag="rsa",
                           name=f"rsa{dname}l{l}")
            agT = eta.tile([128, NBLK, 128], BF, tag="agT",
                           name=f"agT{dname}l{l}")

            def block_tail(b):
                # inline normalize+gelu(tanh)+update for one dst block
                gnt = ew.tile([128, 128], BF, tag="gn")
                tt(nc.vector, gnt[:].rearrange("p (d h) -> p d h", h=8),
                   agf[:, b, :].rearrange("p (d h) -> p d h", h=8),
                   rsa[:, b, :].rearrange("p (d2 h) -> p d2 h", d2=1), ALU.mult)
                t2 = ew.tile([128, 128], BF, tag="t2")
                nc.scalar.square(t2[:], gnt[:])
                u = ew.tile([128, 128], BF, tag="u")
                nc.vector.tensor_scalar(u[:], t2[:], 0.044715, 1.0,
                                        ALU.mult, ALU.add)
                v = ew.tile([128, 128], BF, tag="v")
                tt(nc.vector, v[:], gnt[:], u[:], ALU.mult)
                th = ew.tile([128, 128], BF, tag="th")
                nc.scalar.activation(th[:], v[:], AF.Tanh,
                                     scale=0.7978845608)
                hm = ew.tile([128, 128], BF, tag="hm")
                nc.vector.tensor_scalar(hm[:], th[:], 0.5, 0.5,
                                        ALU.mult, ALU.add)
                gb = ew.tile([128, 128], BF, tag="gb")
                tt(nc.vector, gb[:], gnt[:], hm[:], ALU.mult)
                trp = epsu.tile([128, 128], BF, tag="trp")
                nc.tensor.transpose(trp[:], gb[:], W["ident"])
                gT = ew.tile([128, 128], BF, tag="gT")
                nc.scalar.copy(gT[:], trp[:])
                psU = epsu.tile([128, 128], dt.float32, tag="psU")
                nc.tensor.matmul(psU[:], gT[:], W[f"Wup{dtt}_l{l}"],
                                 start=True, stop=False)
                if not bias_zero:
                    nc.tensor.matmul(psU[:], W["ones1"][:1, :],
                                     W[f"bup{dtt}_l{l}"][:1, :],
                                     start=False, stop=False)
                nc.tensor.matmul(psU[:], W[f"Ibl{dtt}_l{l}"],
                                 xrow[dtt][:, b, :], start=False, stop=True)
                nc.scalar.copy(xnew[dtt][:, b, :], psU[:])
                if do_cc and l == 0:
                    trpn = epsu.tile([128, 128], BF, tag="trp")
                    nc.tensor.transpose(trpn[:], xnew[dtt][:, b, :],
                                        W["ident"])
                    cpy(nc.scalar if b % 2 == 0 else nc.vector,
                        agT[:, b, :], trpn[:])

            def write_agin(b0, b1):
                # stage this half's cc input early; the collective itself is
                # emitted later so it never holds Pool SEQ waiting for input
                if not do_cc:
                    return
                if l == 0:
                    h = "a" if b0 else "b"
                    nc.sync.dma_start(aginTh[f"{dtt}{h}"][:, :],
                                      agT[:, b0:b1, :].rearrange(
                                          "p b c -> p (b c)"))
                else:
                    nc.sync.dma_start(
                        agin2b.rearrange("(b p) c -> p b c", p=128)[:, b0:b1, :],
                        xnew[dtt][:, b0:b1, :])

            def cc_only(h, b0, b1):
                if not do_cc:
                    return
                if l == 0:
                    nc.gpsimd.collective_compute(
                        "AllGather", mybir.AluOpType.bypass,
                        ins=[aginTh[f"{dtt}{h}"][:, :]],
                        outs=[agoutT[f"{dtt}{h}"][:]],
                        replica_groups=[list(range(NCORE))])
                elif h == "b":
                    # layer 1: one full collective (gates the final phase;
                    # splitting it only adds fixed overhead)
                    nc.gpsimd.collective_compute(
                        "AllGather", mybir.AluOpType.bypass,
                        ins=[agin2b[:]], outs=[agout2bf[:]],
                        replica_groups=[list(range(NCORE))])

            done = 0
            for b in border:
                J = Js[b]
                nch = (J + CH - 1) // CH
                psG = sacc = None
                for ci in range(nch):
                    j0 = ci * CH
                    jn = min(CH, J - j0)
                    first = ci == 0
                    last = ci == nch - 1
                    coff8 = off8[b] + j0 * 8
                    kvg = eg.tile([128, CH, 256], BF, tag="kvg")
                    gat(kvg, kv[st], W[f"ei{dname}"], coff8, jn, 256)
                    prod = epw.tile([128, CH, 128], BF, tag="pw", name="prod")
                    tt(nc.vector, prod[:, 0:jn, :], kvg[:, 0:jn, 0:128],
                       qmy[dtt][:, b:b + 1, :], ALU.mult)
                    # alpha[p, j, h] = sum_d prod[p, j, d*8+h]: PE
                    # identity-matmul accumulation over the 16 d-slabs
                    psA = eps.tile([128, CH * 8], dt.float32, tag="psA")
                    for dd in range(D):
                        nc.tensor.matmul(psA[:, 0:jn * 8], W["ident"],
                                         prod[:, 0:jn, bass.ts(dd, 8)],
                                         start=(dd == 0), stop=(dd == D - 1))
                    eB = ew.tile([128, CH, 8], BF, tag="eB")
                    nc.scalar.activation(
                        eB[:, 0:jn, :],
                        psA[:, 0:jn * 8].rearrange("p (j h) -> p j h", h=8),
                        AF.Exp)
                    # wv[p, j, d*8+h] = v * e  (2x: d-major v, h innermost)
                    wv = epw.tile([128, CH, 128], BF, tag="pw", name="wv")
                    tt(nc.vector,
                       wv[:, 0:jn, :].rearrange("p j (d h) -> p j d h", h=8),
                       kvg[:, 0:jn, 128:256].rearrange("p j (d h) -> p j d h", h=8),
                       eB[:, 0:jn, :].rearrange("p j (d2 h) -> p j d2 h", d2=1),
                       ALU.mult)
                    # s tree over j (in-place on eB), bf16 accum
                    with nc.allow_low_precision(reason="softmax denom bf16 tree"):
                        Jc = jn
                        while Jc > 1:
                            h1 = (Jc + 1) // 2
                            tt(nc.vector, eB[:, 0:Jc - h1, :], eB[:, 0:Jc - h1, :],
                               eB[:, h1:Jc, :], ALU.add)
                            Jc = h1
                    if first:
                        sacc = ew.tile([128, 8], dt.float32, tag="sacc")
                        nc.vector.tensor_copy(sacc[:], eB[:, 0, :])
                    else:
                        tt(nc.vector, sacc[:], sacc[:], eB[:, 0, :], ALU.add)
                    # agg[p, dh] += sum_j wv: 4-tile-packed identity matmuls
                    if first:
                        psG = epsg.tile([128, 4, 128], dt.float32, tag="psG")
                    nst = (jn + 3) // 4
                    for g in range(nst):
                        gw = min(4, jn - g * 4)
                        nc.tensor.matmul(psG[:, 0:gw, :], W["ident"],
                                         wv[:, g * 4:g * 4 + gw, :],
                                         start=(first and g == 0),
                                         stop=(last and g == nst - 1))
                    if not last:
                        continue
                    sden = ew.tile([128, 8], dt.float32, tag="sden")
                    tt(nc.vector, sden[:, :], sacc[:],
                       W[f"pc{dname}"][:, b:b + 1], ALU.subtract)
                    nc.vector.reciprocal(rsa[:, b, :], sden[:])
                    nc.vector.tensor_reduce(
                        agf[:, b, :], psG[:].rearrange("p r c -> p c r"),
                        mybir.AxisListType.X, ALU.add)
                block_tail(b)
                done += 1
                if done == 2 and early is not None:
                    early()
                if done == NBLK // 2:
                    write_agin(NBLK // 2, NBLK)
                    if mid is not None:
                        mid()
                if done == NBLK // 2 + 1 and l == 0:
                    # input staged a block ago -> negligible Pool SEQ hold
                    cc_only("a", NBLK // 2, NBLK)
                if done == NBLK - 1 and mid2 is not None:
                    mid2()
            write_agin(0, NBLK // 2)
            if dtt == 1:
                # my updated type-1 rows into the local final table
                nc.sync.dma_start(
                    xtloc.rearrange("(b p) c -> p b c", p=128)[:, :, l * 128:(l + 1) * 128],
                    xnew[1][:])
            return (lambda: cc_only("b", 0, NBLK // 2)) if do_cc else None

        def post_xc(t):
            # xc[t] column order IS the AllGather row order, so the rebuild
            # is 16 plain contiguous reads straight into the xc tile
            for h, b0 in (("a", NBLK // 2), ("b", 0)):
                for r in range(NCORE):
                    nc.sync.dma_start(
                        xc[t][:, r * NLOC + b0 * 128:
                              r * NLOC + b0 * 128 + HB],
                        agoutT[f"{t}{h}"][r])

        def emed_l0_from_xc():
            # final ed table's layer-0 columns = node-major transposes of
            # xc2 (runs in layer-1 slack, off every critical path)
            with tc.tile_pool(name="eml0", bufs=3) as pp, \
                 tc.tile_pool(name="eml0ps", bufs=2, space="PSUM") as ppp:
                emv = emed2c[0:NCORE * NLOC, 0:128].rearrange(
                    "(g p) c -> p g c", p=128)
                for i4 in range(NT // 4):
                    ptr = ppp.tile([128, 4, 128], BF, tag="ptr")
                    for k in range(4):
                        nc.tensor.transpose(
                            ptr[:, k, :],
                            xc[2][:, bass.ts(i4 * 4 + k, 128)], W["ident"])
                    rstage = pp.tile([128, 4, 128], BF, tag="rst")
                    cpy(nc.scalar if i4 % 2 == 0 else nc.vector,
                        rstage[:], ptr[:])
                    nc.sync.dma_start(emv[:, i4 * 4:(i4 + 1) * 4, :],
                                      rstage[:])

        def copy_l1_half():
            # stage agout2bf into the final ed table's layer-1 columns
            with tc.tile_pool(name="cl1", bufs=3) as pp:
                emv = emed2c[0:NCORE * NLOC, 128:256].rearrange(
                    "(g p) c -> p g c", p=128)
                srcv = agout2bf[:].rearrange("r (b p) c -> p (r b) c",
                                             p=128)
                for q4 in range(NT // 4):
                    rd = pp.tile([128, 4, 128], BF, tag="rd")
                    nc.sync.dma_start(rd[:], srcv[:, q4 * 4:(q4 + 1) * 4, :])
                    nc.sync.dma_start(emv[:, q4 * 4:(q4 + 1) * 4, :], rd[:])

        # ---------- layers: phase stamps act as scheduler barriers; keep
        # one only where collective-gated work could otherwise poison an
        # independent engine stream ----------
        tc.tile_set_cur_wait(1)
        make_xc(2)
        emit_tables(2, 0, psbufs=4)
        make_xc(1)
        make_xt0(1)
        xrow_gather(1)
        qmy_local(1, 0)
        p0stack.close()
        open_edge_pools()
        tc.tile_set_cur_wait(2)
        emit_tables(1, 0)
        cc1b = edge_dir("21", 2, 1, 0,
                        mid=lambda: make_xt0(2),
                        mid2=lambda: (xrow_gather(2), qmy_local(2, 0)))
        cc2b = edge_dir("12", 1, 2, 0, early=cc1b)
        xrow, xnew = xnew, xrow
        # the cc-gated packs stay in the PREVIOUS phase: their gates
        # complete mid-direction, so they fill the direction's drain
        cc2b()
        post_xc(1)
        emit_tables(1, 1)
        qmy_local(2, 1)
        qmy_local(1, 1)
        tc.tile_set_cur_wait(3)
        cc2bb = edge_dir("12", 1, 2, 1)
        cc2bb()
        post_xc(2)
        emit_tables(2, 1)
        emed_l0_from_xc()
        tc.tile_set_cur_wait(4)
        edge_dir("21", 2, 1, 1, do_cc=False)
        copy_l1_half()
        tc.tile_set_cur_wait(5)
        estack.close()

        # ---------- final gather-dot (m-grouped, local dense Em side) ----------
        with tc.tile_pool(name="fin", bufs=4) as fp, \
             tc.tile_pool(name="fpb", bufs=3) as fpb, \
             tc.tile_pool(name="fem", bufs=1) as fem, \
             tc.tile_pool(name="finps", bufs=4, space="PSUM") as fps, \
             tc.tile_pool(name="ybuf", bufs=1) as yp:
            ysb = yp.tile([128, YC], dt.float32, tag="ysb")
            emT = fem.tile([128, NFT, 256], BF, tag="emT")
            gat(emT, xtloc, W["emlid"], 0, NFT, 256)
            col = 0
            for g in range(NFT):
                base8 = sum(JF[:g]) * 8
                for j0 in range(0, JF[g], 8):
                    gn_t = min(8, JF[g] - j0)
                    ed = fp.tile([128, 8, 256], BF, tag="ed")
                    gat(ed, emed2c, W["fei"], base8 + j0 * 8, gn_t, 256)
                    pb = fpb.tile([128, 8, 256], BF, tag="pb")
                    tt(nc.vector, pb[:, 0:gn_t, :], ed[:, 0:gn_t, :],
                       emT[:, g:g + 1, :], ALU.mult)
                    # slab-sum on PE: psY[p, t, i] = sum_s pb[p, t, s*16+i]
                    psY = fps.tile([128, 8, 16], dt.float32, tag="psY")
                    for s in range(16):
                        nc.tensor.matmul(psY[:, 0:gn_t, :], W["ident"],
                                         pb[:, 0:gn_t, bass.ts(s, 16)],
                                         start=(s == 0), stop=(s == 15))
                    nc.vector.tensor_reduce(
                        ysb[:, col:col + gn_t], psY[:, 0:gn_t, :],
                        mybir.AxisListType.X, ALU.add)
                    col += gn_t
            nc.sync.dma_start(y_out[:, :], ysb[:])
    nc.compile()
    return nc


_CACHE = {}
_last_key = None


def kernel(**inputs):
    global _last_key
    from concourse.bass_utils import run_bass_kernel_spmd
    P, e12, e21, fin, J12, J21, JF = _host_prep(inputs)
    bz = all(not np.any(np.asarray(inputs[k]))
             for k in inputs if k.startswith("b"))
    key = (J12, J21, JF, bz)
    _last_key = key
    if key not in _CACHE:
        _CACHE[key] = _build(J12, J21, JF, bias_zero=bz)
    nc = _CACHE[key]
    in_maps = []
    for c in range(NCORE):
        m = dict(P)
        m["ei12"] = e12[c]["idx"]; m["pc12"] = e12[c]["padc"]
        m["ei21"] = e21[c]["idx"]; m["pc21"] = e21[c]["padc"]
        m["myid"] = fin[c]["myid"]
        m["emlid"] = fin[c]["emlid"]; m["fei"] = fin[c]["fei"]
        in_maps.append(m)
    res = run_bass_kernel_spmd(nc, in_maps, list(range(NCORE)))
    y = np.zeros((EF,), np.float32)
    for c in range(NCORE):
        yc = np.asarray(res.results[c]["y"])      # [128, YC]
        ylin = yc.T.ravel()                        # slot (col, p) order
        pos = fin[c]["pos"]
        mreal = pos >= 0
        y[pos[mreal]] = ylin[mreal]
    return y.reshape(EF, 1)
